# revision 1
# baseline (speedup 1.0000x reference)
"""DKT model (2-layer LSTM + FC + sigmoid) as a Bass/Tile kernel for 8
Trainium2 NeuronCores, data-parallel over the batch dim (64 -> 8 per core).

Structure per core (everything "transposed": hidden/gate index on SBUF
partitions, batch on the free dim, so ACT/DVE use all 128 lanes):

  - One-hot @ W_ih0 is an embedding lookup: gather rows of
    (W_ih0 + b_ih0 + b_hh0)^T from a DRAM table via indirect DMA,
    DMA-transpose to gate-major layout, spill to DRAM, stream back
    during the recurrence.
  - LSTM recurrence (v2, default): gates accumulate in PSUM over 64
    [128x128]x[128x8] weight-stationary matmuls; x is added into PSUM in
    place (one DVE op); ALL FOUR gates go through a single merged ACT
    sigmoid straight out of PSUM (tanh(x) = 2*sigmoid(2x)-1, with the 2x
    folded into the g-gate rows of the weights host-side); the cell
    update is 6 small DVE ops; h lands in a local tile whose slots feed
    the next step's matmul rhs (static APs only on the PE).
  - The two layers run software-pipelined at half-chunk (256-token,
    32-step) granularity: iteration q interleaves L0 steps of half q
    with L1 steps of half q-1 per step, so each layer's serial
    ACT/DVE tail hides under the other layer's matmul stream.
  - Layer-1 input matmul X1 = h1 @ W_ih1^T + b1 batched per half chunk
    straight from the L0 local h tile at body end (N=256, static APs;
    no h1-history round trip); FC + sigmoid batched at the end, stored
    to DRAM in PE-native layout with large contiguous DMAs (the host
    untangles [g_lo, nchunk, m, tok] -> [b, s, g]; the naive transposed
    store degenerated to 4-byte DMA packets and serialized the kernel).
  Measured on HW: 4.43ms (from 54.9ms baseline). Known remaining gap:
  the Tile scheduler statically interleaves both layers' DVE chains, so
  the chain the next PE burst depends on finishes ~1us late per pair
  (~8.6us/pair vs ~5us ideal); forcing strict chain order with bufs=1
  scratch tiles made it WORSE (5.18ms) - each chain then waits out the
  other's full ACT round-trip latency.

Env knobs (defaults are the fast path): DKT_V=1 selects the legacy
build, DKT_IL=0 disables the layer interleave, DKT_WDT=fp8 switches the
recurrent weights to fp8 e4m3 with x64 scaling, DKT_S shortens the
sequence for dev testing, DKT_FASTPROF=1 captures an NTFF and prints
exec time via neuron-profile summary (dev only; needs prof_shim.py).
"""
import os
import numpy as np
import ml_dtypes

import concourse.bass as bass
import concourse.mybir as mybir
import concourse.tile as tile
import concourse.tile as tile_mod
from concourse.bass import ds
from concourse.vector_clock import ScopedClock

BF16 = mybir.dt.bfloat16
FP8 = mybir.dt.float8e4
F32 = mybir.dt.float32
I32 = mybir.dt.int32
AF = mybir.ActivationFunctionType
ALU = mybir.AluOpType

np_bf16 = ml_dtypes.bfloat16
np_fp8 = ml_dtypes.float8_e4m3

P = 128
H = 512          # hidden
G = 2048         # 4*H gates
NSK = 1024       # n_skills
S = int(os.environ.get('DKT_S', '512'))  # seq len
B = 64           # full batch
NCORES = 8
BC = B // NCORES          # batch per core = 8
T = S * BC                # tokens per core = 4096
TC = P                    # tokens per gather chunk (= 16 steps)
NCHUNK = T // TC          # 32
KCH = 4                   # K chunks of 128 over hidden 512
MCH = 16                  # M chunks of 128 over gates 2048

# ---------------------------------------------------------------------------
# Toolchain workarounds (this walrus build rejects >1 semaphore wait per
# instruction, and Tile's exit drain / loop machinery emit several).

_MAX_WAITS = 1
_MAX_INC = 16
_wctr = [0]


def _split_waits(nc):
    for f in nc.m.functions:
        for bb in f.blocks:
            insts = bb.instructions
            i = 0
            while i < len(insts):
                inst = insts[i]
                si = inst.sync_info
                if si is not None and len(si.on_wait) > _MAX_WAITS:
                    waits = list(si.on_wait)
                    keep = waits[-_MAX_WAITS:]
                    extra = waits[:-_MAX_WAITS]
                    inst.sync_info = mybir.SyncInfo(
                        on_wait=keep, on_update=list(si.on_update)
                    )
                    si = inst.sync_info
                    pos = i
                    for j in range(0, len(extra), _MAX_WAITS):
                        _wctr[0] += 1
                        nop = mybir.InstNoOp(
                            name=f"wsplit_{_wctr[0]}", engine=inst.engine
                        )
                        nop.sync_info = mybir.SyncInfo(
                            on_wait=extra[j : j + _MAX_WAITS], on_update=[]
                        )
                        insts.insert(pos, nop)
                        pos += 1
                        i += 1
                # Non-NoOp instructions reject large sem-update immediates in
                # this walrus; move those updates intact onto an adjacent NoOp
                # (Tile's own stage-reset NoOps carry 1000+ values fine).
                # Never alter values: sem-wr-imm is an absolute write.
                # Same story for an update to a semaphore the instruction also
                # waits on (walrus 'no_semaphore_value_conflict').
                # DMA-ish instructions legitimately wait on and bump the same
                # FIFO semaphore (and need their completion update in place);
                # only compute ops trip the conflict check.
                is_compute = isinstance(
                    inst,
                    (
                        mybir.InstActivation,
                        mybir.InstTensorTensor,
                        mybir.InstTensorScalarPtr,
                        mybir.InstMatmult,
                        mybir.InstMemset,
                        mybir.InstTensorCopy,
                    ),
                )
                def _bad_update(u):
                    if (u.update_value or 0) > _MAX_INC:
                        return True
                    # compute ops only encode plain sem-inc; add-imm etc.
                    # must ride on a NoOp ('no_semaphore_value_conflict')
                    return is_compute and str(u.update_mode) != "sem-inc"

                if (
                    si is not None
                    and not isinstance(inst, mybir.InstNoOp)
                    and any(_bad_update(u) for u in si.on_update)
                ):
                    keep_ups, extras = [], []
                    for u in si.on_update:
                        if _bad_update(u):
                            extras.append(u)
                        else:
                            keep_ups.append(u)
                    inst.sync_info = mybir.SyncInfo(
                        on_wait=list(si.on_wait), on_update=keep_ups
                    )
                    is_branch = isinstance(
                        inst,
                        (
                            mybir.InstCompareAndBranch,
                            mybir.InstUnconditionalBranch,
                        ),
                    )
                    pos = i if is_branch else i + 1
                    for u in extras:
                        _wctr[0] += 1
                        nop = mybir.InstNoOp(
                            name=f"isplit_{_wctr[0]}", engine=inst.engine
                        )
                        nop.sync_info = mybir.SyncInfo(on_wait=[], on_update=[u])
                        insts.insert(pos, nop)
                        pos += 1
                        if is_branch:
                            i += 1
                i += 1


def _patched_drain_and_barrier(self, tick_clock, wait_clock):
    nc = self.nc
    drain_inst = nc.sync.drain()
    wait_clock.add_sem_waits(
        drain_inst.ins, ScopedClock({None: tick_clock.global_clock})
    )
    si = drain_inst.ins.sync_info
    if si is not None and len(si.on_wait) > 1:
        waits = list(si.on_wait)
        drain_inst.ins.sync_info = mybir.SyncInfo(
            on_wait=waits[:1], on_update=list(si.on_update)
        )
        for k in range(1, len(waits)):
            nop = nc.sync.nop()
            nop.ins.sync_info = mybir.SyncInfo(on_wait=[waits[k]], on_update=[])
    nc.all_engine_barrier()
    assert self.sems is not None
    popped = nc._tile_sem_poison_stack.pop()
    assert popped is self._sem_poison
    nc.clear_and_free_semaphores(list(self.sems.allocated().values()))
    nc.all_engine_barrier()


tile_mod.TileContext._drain_and_barrier = _patched_drain_and_barrier

# ---------------------------------------------------------------------------


def _build(use_fp8: bool, split: bool = True):
    wdt = FP8 if use_fp8 else BF16
    scale = 64.0 if use_fp8 else 1.0
    inv_scale = 1.0 / scale

    nc = bass.Bass()
    table_d = nc.dram_tensor("table", [G, G], BF16, kind="ExternalInput")
    wh0_d = nc.dram_tensor("wh0", [P, KCH, G], wdt, kind="ExternalInput")
    wh1_d = nc.dram_tensor("wh1", [P, KCH, G], wdt, kind="ExternalInput")
    wi1_d = nc.dram_tensor("wi1", [P, KCH, G], BF16, kind="ExternalInput")
    fcw_d = nc.dram_tensor("fcw", [P, KCH, NSK], BF16, kind="ExternalInput")
    b1_d = nc.dram_tensor("b1", [P, MCH], F32, kind="ExternalInput")
    fcb_d = nc.dram_tensor("fcb", [P, NSK // P], F32, kind="ExternalInput")
    sk_d = nc.dram_tensor("skills", [BC, S], I32, kind="ExternalInput")
    co_d = nc.dram_tensor("corrects", [BC, S], I32, kind="ExternalInput")
    # Output in PE-native layout [g_lo, nchunk, m, tok]; host untangles it.
    # (The old [b, s, g] transposed store degenerated to 4-byte DMA packets
    # and serialized the whole kernel on the sync DMA ring.)
    NT = T // 512
    NM = NSK // P
    out_d = nc.dram_tensor("out", [P, NT, NM, 512], BF16, kind="ExternalOutput")

    with tile.TileContext(nc) as tc:
        with tc.tile_pool(name="wpool", bufs=1) as wpool, \
             tc.tile_pool(name="hist", bufs=1) as hist, \
             tc.tile_pool(name="work", bufs=1) as work, \
             tc.tile_pool(name="dram", bufs=1, space="DRAM") as dpool, \
             tc.tile_pool(name="x0pool", bufs=1) as x0pool, \
             tc.tile_pool(name="psrec", bufs=2, space="PSUM") as psrec, \
             tc.tile_pool(name="psbig", bufs=2, space="PSUM") as psbig:

            # ---- resident weights ----
            wh0_sb = wpool.tile([P, KCH, G], wdt)
            nc.sync.dma_start(wh0_sb[:], wh0_d[:])
            wh1_sb = wpool.tile([P, KCH, G], wdt)
            nc.sync.dma_start(wh1_sb[:], wh1_d[:])
            wi1_sb = wpool.tile([P, KCH, G], BF16)
            nc.sync.dma_start(wi1_sb[:], wi1_d[:])
            fcw_sb = wpool.tile([P, KCH, NSK], BF16)
            nc.sync.dma_start(fcw_sb[:], fcw_d[:])
            b1_sb = wpool.tile([P, MCH], F32)
            nc.sync.dma_start(b1_sb[:], b1_d[:])
            fcb_sb = wpool.tile([P, NSK // P], F32)
            nc.sync.dma_start(fcb_sb[:], fcb_d[:])

            # ---- token-major idx = skills + 1024*(1 - corrects) ----
            sk_sb = work.tile([P, NCHUNK], I32)
            nc.sync.dma_start(
                sk_sb[:], sk_d[:].rearrange("b (c t) -> t b c", t=TC // BC))
            co_sb = work.tile([P, NCHUNK], I32)
            nc.sync.dma_start(
                co_sb[:], co_d[:].rearrange("b (c t) -> t b c", t=TC // BC))
            idx_sb = work.tile([P, NCHUNK], I32)
            nc.vector.tensor_scalar(
                idx_sb[:], co_sb[:], -1024, 1024, ALU.mult, ALU.add
            )
            nc.vector.tensor_tensor(
                out=idx_sb[:], in0=idx_sb[:], in1=sk_sb[:], op=ALU.add
            )

            # ---- gather + transpose + spill X0T to DRAM (static) ----
            x0t_dram = dpool.tile([NCHUNK + 2, P, MCH, TC], BF16)
            rows = [work.tile([P, G], BF16, name=f"rows{u}") for u in range(2)]
            x0st = [work.tile([P, MCH, TC], BF16, name=f"x0st{u}") for u in range(2)]
            for ch in range(NCHUNK):
                u = ch % 2
                nc.gpsimd.indirect_dma_start(
                    out=rows[u][:], out_offset=None, in_=table_d[:],
                    in_offset=bass.IndirectOffsetOnAxis(
                        ap=idx_sb[:, ch : ch + 1], axis=0
                    ),
                )
                for m in range(MCH):
                    nc.sync.dma_start_transpose(
                        x0st[u][:, m, :], rows[u][:, m * P : (m + 1) * P]
                    )
                nc.sync.dma_start(x0t_dram[ch], x0st[u][:])
            # pad chunks (read by the tail prefetch, never consumed)
            zpad = work.tile([P, MCH, TC], BF16)
            nc.vector.memset(zpad[:], 0.0)
            nc.sync.dma_start(x0t_dram[NCHUNK], zpad[:])
            nc.sync.dma_start(x0t_dram[NCHUNK + 1], zpad[:])

            # ---- histories (token-major, SBUF-resident) ----
            h1t = hist.tile([P, KCH, T], BF16)
            h2t = hist.tile([P, KCH, T], BF16)

            c_ab = [work.tile([P, KCH * BC], F32, name=f"c{u}") for u in range(2)]

            def lstm_step(w_sb, x_ap, h_loc, st, cprev, cnew, sname):
                """One step: gates = x + W_hh@h_prev; update c, h.
                x_ap: [P, MCH, BC] (pre-scaled by `scale`, bias folded);
                h_loc: local history [P, KCH, nsteps*BC + BC] with the
                previous body's last h in slots [0:BC]; step `st` reads
                slots [st*BC : st*BC+BC], writes the next BC."""
                psum = psrec.tile([P, MCH * BC], F32, name=f"ps_{sname}", tag="recps")
                for m in range(MCH):
                    for k in range(KCH):
                        nc.tensor.matmul(
                            psum[:, m * BC : (m + 1) * BC],
                            lhsT=w_sb[:, k, m * P : (m + 1) * P],
                            rhs=h_loc[:, k, st * BC : (st + 1) * BC],
                            start=(k == 0),
                            stop=(k == KCH - 1),
                        )
                gp = work.tile([P, MCH * BC], F32, name=f"gp_{sname}", tag="gp", bufs=2)
                nc.vector.tensor_tensor(
                    out=gp[:].rearrange("p (m b) -> p m b", b=BC),
                    in0=psum[:].rearrange("p (m b) -> p m b", b=BC),
                    in1=x_ap,
                    op=ALU.add,
                )
                ac = work.tile([P, MCH * BC], F32, name=f"ac_{sname}", tag="ac", bufs=2)
                nc.scalar.activation(ac[:, 0:64], gp[:, 0:64], AF.Sigmoid,
                                     scale=inv_scale)
                nc.scalar.activation(ac[:, 64:96], gp[:, 64:96], AF.Tanh,
                                     scale=inv_scale)
                nc.scalar.activation(ac[:, 96:128], gp[:, 96:128], AF.Sigmoid,
                                     scale=inv_scale)
                t1 = work.tile([P, KCH * BC], F32, name=f"t1_{sname}", tag="t1", bufs=2)
                nc.vector.tensor_tensor(out=t1[:], in0=ac[:, 32:64], in1=cprev[:],
                                        op=ALU.mult)
                t2 = work.tile([P, KCH * BC], F32, name=f"t2_{sname}", tag="t2", bufs=2)
                nc.vector.tensor_tensor(out=t2[:], in0=ac[:, 0:32], in1=ac[:, 64:96],
                                        op=ALU.mult)
                nc.vector.tensor_tensor(out=cnew[:], in0=t1[:], in1=t2[:], op=ALU.add)
                th = work.tile([P, KCH * BC], F32, name=f"th_{sname}", tag="th", bufs=2)
                nc.scalar.activation(th[:], cnew[:], AF.Tanh)
                nc.vector.tensor_tensor(
                    out=h_loc[:, :, (st + 1) * BC : (st + 2) * BC],
                    in0=ac[:, 96:128].rearrange("p (k b) -> p k b", b=BC),
                    in1=th[:].rearrange("p (k b) -> p k b", b=BC),
                    op=ALU.mult,
                )

            # ---- layer 0 recurrence: 16 iterations x 2 chunks x 16 steps ----
            nc.vector.memset(c_ab[0][:], 0.0)
            NST0 = 32  # steps per body
            h0_loc = work.tile([P, KCH, (NST0 + 1) * BC], BF16)
            nc.vector.memset(h0_loc[:, :, 0:BC], 0.0)
            x0t = [x0pool.tile([P, MCH, TC], BF16, name=f"x0t{u}") for u in range(2)]
            nc.sync.dma_start(x0t[0][:], x0t_dram[0])
            with tc.For_i(0, NCHUNK // 2, 1, staggered_reset=True) as i:
                nc.sync.dma_start(x0t[1][:], x0t_dram[ds(2 * i + 1, 1)]
                                  .rearrange("a p m t -> (a p) m t"))
                for half in range(2):
                    for u in range(16):
                        st = half * 16 + u  # step within the 2-chunk body
                        lstm_step(
                            wh0_sb,
                            x0t[half][:, :, u * BC : (u + 1) * BC],
                            h0_loc, st,
                            c_ab[st % 2], c_ab[(st + 1) % 2],
                            f"l0_{half}_{u}",
                        )
                nc.sync.dma_start(x0t[0][:], x0t_dram[ds(2 * i + 2, 1)]
                                  .rearrange("a p m t -> (a p) m t"))
                # flush this body's h into the history + carry last h to slot 0
                nc.vector.tensor_copy(
                    h1t[:, :, ds(i * (NST0 * BC), NST0 * BC)],
                    h0_loc[:, :, BC : (NST0 + 1) * BC],
                )
                nc.vector.tensor_copy(
                    h0_loc[:, :, 0:BC], h0_loc[:, :, NST0 * BC : (NST0 + 1) * BC]
                )

            # ---- layer 1: per 64-step chunk, batched X1 then recurrence ----
            nc.vector.memset(c_ab[0][:], 0.0)
            TJ = 512  # tokens per L1 chunk
            NST1 = TJ // BC  # 64 steps per body
            h1_loc = work.tile([P, KCH, (NST1 + 1) * BC], BF16)
            nc.vector.memset(h1_loc[:, :, 0:BC], 0.0)
            x1t = x0pool.tile([P, MCH, TJ], BF16)
            hx = work.tile([P, KCH, TJ], BF16)
            with tc.For_i(0, T // TJ, 1, staggered_reset=True) as j:
                jb = nc.snap(j * TJ)
                nc.vector.tensor_copy(hx[:], h1t[:, :, ds(jb, TJ)])
                for m in range(MCH):
                    psx = psbig.tile([P, TJ], F32, name=f"psx{m}", tag="psx")
                    for k in range(KCH):
                        nc.tensor.matmul(
                            psx[:],
                            lhsT=wi1_sb[:, k, m * P : (m + 1) * P],
                            rhs=hx[:, k, :],
                            start=(k == 0),
                            stop=(k == KCH - 1),
                        )
                    nc.scalar.activation(x1t[:, m, :], psx[:], AF.Identity,
                                         bias=b1_sb[:, m : m + 1], scale=scale)
                for u in range(NST1):
                    lstm_step(
                        wh1_sb,
                        x1t[:, :, u * BC : (u + 1) * BC],
                        h1_loc, u,
                        c_ab[u % 2], c_ab[(u + 1) % 2],
                        f"l1_{u}",
                    )
                nc.vector.tensor_copy(
                    h2t[:, :, ds(jb, TJ)],
                    h1_loc[:, :, BC : (NST1 + 1) * BC],
                )
                nc.vector.tensor_copy(
                    h1_loc[:, :, 0:BC], h1_loc[:, :, NST1 * BC : (NST1 + 1) * BC]
                )

            # ---- FC + sigmoid + store (static) ----
            for n in range(NT):
                obuf = work.tile([P, NM, 512], BF16, name=f"ob{n}", tag="ob",
                                 bufs=2)
                for m in range(NM):
                    psf = psbig.tile([P, 512], F32, name=f"psf{n}_{m}", tag="psf")
                    for k in range(KCH):
                        nc.tensor.matmul(
                            psf[:],
                            lhsT=fcw_sb[:, k, m * P : (m + 1) * P],
                            rhs=h2t[:, k, n * 512 : (n + 1) * 512],
                            start=(k == 0),
                            stop=(k == KCH - 1),
                        )
                    nc.scalar.activation(obuf[:, m, :], psf[:], AF.Sigmoid,
                                         bias=fcb_sb[:, m : m + 1])
                nc.sync.dma_start(out_d[:, n], obuf[:])

    if split:
        _split_waits(nc)
    return nc


def _build2(use_fp8: bool, split: bool = True):
    """v2: restructured recurrence.

    - all four gates through ONE merged sigmoid per step (tanh(x) =
      2*sigmoid(2x)-1; the 2x is folded into the g-gate rows of the
      weights/biases host-side), ACT reads gates straight from PSUM
    - x added into PSUM in place (single DVE op), no gp/ac/t1/t2/th tiles
    - h histories (h1t/h2t) double as the recurrence rhs via dynamic
      offsets: no h_loc, no history flush copies
    - gather scratch in a short-lived pool (SBUF reclaimed)
    - one For_i over 8 chunks of 64 steps per layer
    """
    wdt = FP8 if use_fp8 else BF16
    scale = 64.0 if use_fp8 else 1.0
    inv_scale = 1.0 / scale

    TJ = 512                  # tokens per chunk
    NST = TJ // BC            # 64 steps per chunk
    NCH = T // TJ             # 8 chunks
    GPC = TJ // TC            # 4 gather chunks per chunk

    nc = bass.Bass()
    table_d = nc.dram_tensor("table", [G, G], BF16, kind="ExternalInput")
    wh0_d = nc.dram_tensor("wh0", [P, KCH, G], wdt, kind="ExternalInput")
    wh1_d = nc.dram_tensor("wh1", [P, KCH, G], wdt, kind="ExternalInput")
    wi1_d = nc.dram_tensor("wi1", [P, KCH, G], BF16, kind="ExternalInput")
    fcw_d = nc.dram_tensor("fcw", [P, KCH, NSK], BF16, kind="ExternalInput")
    b1_d = nc.dram_tensor("b1", [P, MCH], F32, kind="ExternalInput")
    fcb_d = nc.dram_tensor("fcb", [P, NSK // P], F32, kind="ExternalInput")
    sk_d = nc.dram_tensor("skills", [BC, S], I32, kind="ExternalInput")
    co_d = nc.dram_tensor("corrects", [BC, S], I32, kind="ExternalInput")
    NT = T // 512
    NM = NSK // P
    out_d = nc.dram_tensor("out", [P, NT, NM, 512], BF16, kind="ExternalOutput")

    with tile.TileContext(nc) as tc:
        with tc.tile_pool(name="wpool", bufs=1) as wpool, \
             tc.tile_pool(name="hist", bufs=1) as hist, \
             tc.tile_pool(name="work", bufs=1) as work, \
             tc.tile_pool(name="dram", bufs=1, space="DRAM") as dpool, \
             tc.tile_pool(name="x0pool", bufs=1) as x0pool, \
             tc.tile_pool(name="psrec", bufs=2, space="PSUM") as psrec, \
             tc.tile_pool(name="psbig", bufs=2, space="PSUM") as psbig:

            # ---- resident weights ----
            wh0_sb = wpool.tile([P, KCH, G], wdt)
            nc.sync.dma_start(wh0_sb[:], wh0_d[:])
            wh1_sb = wpool.tile([P, KCH, G], wdt)
            nc.sync.dma_start(wh1_sb[:], wh1_d[:])
            wi1_sb = wpool.tile([P, KCH, G], BF16)
            nc.sync.dma_start(wi1_sb[:], wi1_d[:])
            fcw_sb = wpool.tile([P, KCH, NSK], BF16)
            nc.sync.dma_start(fcw_sb[:], fcw_d[:])
            b1_sb = wpool.tile([P, MCH], F32)
            nc.sync.dma_start(b1_sb[:], b1_d[:])
            fcb_sb = wpool.tile([P, NSK // P], F32)
            nc.sync.dma_start(fcb_sb[:], fcb_d[:])

            # ---- histories (filled chunk-wise from the local h tiles) ----
            h1t = hist.tile([P, KCH, T], BF16)
            h2t = hist.tile([P, KCH, T], BF16)

            # ---- gather + transpose + spill X0T to DRAM (static) ----
            x0t_dram = dpool.tile([NCHUNK + 2 * GPC, P, MCH, TC], BF16)
            with tc.tile_pool(name="gath", bufs=1) as gath:
                sk_sb = gath.tile([P, NCHUNK], I32)
                nc.sync.dma_start(
                    sk_sb[:], sk_d[:].rearrange("b (c t) -> t b c", t=TC // BC))
                co_sb = gath.tile([P, NCHUNK], I32)
                nc.sync.dma_start(
                    co_sb[:], co_d[:].rearrange("b (c t) -> t b c", t=TC // BC))
                idx_sb = gath.tile([P, NCHUNK], I32)
                nc.vector.tensor_scalar(
                    idx_sb[:], co_sb[:], -1024, 1024, ALU.mult, ALU.add
                )
                nc.vector.tensor_tensor(
                    out=idx_sb[:], in0=idx_sb[:], in1=sk_sb[:], op=ALU.add
                )
                rows = [gath.tile([P, G], BF16, name=f"rows{u}")
                        for u in range(2)]
                x0st = [gath.tile([P, MCH, TC], BF16, name=f"x0st{u}")
                        for u in range(2)]
                for ch in range(NCHUNK):
                    u = ch % 2
                    nc.gpsimd.indirect_dma_start(
                        out=rows[u][:], out_offset=None, in_=table_d[:],
                        in_offset=bass.IndirectOffsetOnAxis(
                            ap=idx_sb[:, ch : ch + 1], axis=0
                        ),
                    )
                    for m in range(MCH):
                        nc.sync.dma_start_transpose(
                            x0st[u][:, m, :], rows[u][:, m * P : (m + 1) * P]
                        )
                    nc.sync.dma_start(x0t_dram[ch], x0st[u][:])
                zpad = gath.tile([P, MCH, TC], BF16)
                nc.vector.memset(zpad[:], 0.0)
                for z in range(2 * GPC):
                    nc.sync.dma_start(x0t_dram[NCHUNK + z], zpad[:])

            # ---- per-layer state ----
            c0_ab = [work.tile([P, KCH * BC], F32, name=f"c0{u}")
                     for u in range(2)]
            c1_ab = [work.tile([P, KCH * BC], F32, name=f"c1{u}")
                     for u in range(2)]
            nc.vector.memset(c0_ab[0][:], 0.0)
            nc.vector.memset(c1_ab[0][:], 0.0)

            def lstm_step(w_sb, x_ap, h_loc, u, cprev, cnew, sname,
                          veng=None):
                """gates = W_hh @ h_prev (+x in psum); h -> h_loc slot u+1.

                veng picks the engine for the SBUF-only cell-update ops
                (default DVE). Running one layer's chain on GpSimd keeps
                the two interleaved layers' chains off each other's
                critical path (the PSUM add must stay on DVE: P2)."""
                if veng is None:
                    veng = nc.vector
                ps = psrec.tile([P, MCH * BC], F32, name=f"ps_{sname}",
                                tag="recps")
                for m in range(MCH):
                    for k in range(KCH):
                        nc.tensor.matmul(
                            ps[:, m * BC : (m + 1) * BC],
                            lhsT=w_sb[:, k, m * P : (m + 1) * P],
                            rhs=h_loc[:, k, u * BC : (u + 1) * BC],
                            start=(k == 0),
                            stop=(k == KCH - 1),
                        )
                nc.vector.tensor_tensor(
                    out=ps[:].rearrange("p (m b) -> p m b", b=BC),
                    in0=ps[:].rearrange("p (m b) -> p m b", b=BC),
                    in1=x_ap,
                    op=ALU.add,
                )
                ac = work.tile([P, MCH * BC], F32, name=f"ac_{sname}",
                               tag="ac", bufs=2)
                nc.scalar.activation(ac[:], ps[:], AF.Sigmoid, scale=inv_scale)
                # g' = tanh = 2*sig(2g)-1, in place
                veng.tensor_scalar(ac[:, 64:96], ac[:, 64:96], 2.0, -1.0,
                                   ALU.mult, ALU.add)
                tt = work.tile([P, 2 * KCH * BC], F32, name=f"tt_{sname}",
                               tag="tt", bufs=2)
                veng.tensor_tensor(out=tt[:, 0:32], in0=ac[:, 32:64],
                                   in1=cprev[:], op=ALU.mult)
                veng.tensor_tensor(out=tt[:, 32:64], in0=ac[:, 0:32],
                                   in1=ac[:, 64:96], op=ALU.mult)
                veng.tensor_tensor(out=cnew[:], in0=tt[:, 0:32],
                                   in1=tt[:, 32:64], op=ALU.add)
                sc = work.tile([P, KCH * BC], F32, name=f"sc_{sname}",
                               tag="sc", bufs=2)
                nc.scalar.activation(sc[:], cnew[:], AF.Sigmoid, scale=2.0)
                veng.tensor_scalar(sc[:], sc[:], 2.0, -1.0,
                                   ALU.mult, ALU.add)
                veng.tensor_tensor(
                    out=h_loc[:, :, (u + 1) * BC : (u + 2) * BC],
                    in0=ac[:, 96:128].rearrange("p (k b) -> p k b", b=BC),
                    in1=sc[:].rearrange("p (k b) -> p k b", b=BC),
                    op=ALU.mult,
                )

            # ---- local state + x buffers ----
            HTJ = TJ // 2             # tokens per x0 half-buffer
            h0_loc = work.tile([P, KCH, (NST + 1) * BC], BF16)
            nc.vector.memset(h0_loc[:, :, 0:BC], 0.0)
            h1_loc = work.tile([P, KCH, (NST + 1) * BC], BF16)
            nc.vector.memset(h1_loc[:, :, 0:BC], 0.0)
            interleave = os.environ.get("DKT_IL", "1") == "1"
            if not interleave:
                x0t = [x0pool.tile([P, MCH, HTJ], BF16, name=f"x0t{u}")
                       for u in range(2)]
                x1t = x0pool.tile([P, MCH, TJ], BF16)
                hx = x0pool.tile([P, KCH, TJ], BF16)
                nc.sync.dma_start(
                    x0t[0][:].rearrange("p m (a t) -> p m a t", t=TC),
                    x0t_dram[0:2].rearrange("a p m t -> p m a t"))

            def x0_ap(u):
                if u < NST // 2:
                    return x0t[0][:, :, u * BC : (u + 1) * BC]
                v = u - NST // 2
                return x0t[1][:, :, v * BC : (v + 1) * BC]

            def l0_flush(jb):
                nc.vector.tensor_copy(
                    h1t[:, :, ds(jb, TJ)], h0_loc[:, :, BC : (NST + 1) * BC])
                nc.vector.tensor_copy(
                    h0_loc[:, :, 0:BC],
                    h0_loc[:, :, NST * BC : (NST + 1) * BC])

            def l1_flush(jb):
                nc.vector.tensor_copy(
                    h2t[:, :, ds(jb, TJ)], h1_loc[:, :, BC : (NST + 1) * BC])
                nc.vector.tensor_copy(
                    h1_loc[:, :, 0:BC],
                    h1_loc[:, :, NST * BC : (NST + 1) * BC])

            def x1_batch(jb, tag):
                nc.vector.tensor_copy(hx[:], h1t[:, :, ds(jb, TJ)])
                for m in range(MCH):
                    psx = psbig.tile([P, TJ], F32, name=f"psx_{tag}{m}",
                                     tag="psx")
                    for k in range(KCH):
                        nc.tensor.matmul(
                            psx[:],
                            lhsT=wi1_sb[:, k, m * P : (m + 1) * P],
                            rhs=hx[:, k, :],
                            start=(k == 0),
                            stop=(k == KCH - 1),
                        )
                    if m % 2 == 0:
                        nc.scalar.activation(x1t[:, m, :], psx[:], AF.Identity,
                                             bias=b1_sb[:, m : m + 1],
                                             scale=scale)
                    else:
                        nc.vector.tensor_scalar(
                            x1t[:, m, :], psx[:], scale,
                            b1_sb[:, m : m + 1], ALU.mult, ALU.add)

            if interleave:
                # Half-chunk (HTJ=256 tok, 32 steps) software pipeline:
                # iteration q runs L0 half q and L1 half q-1 step-interleaved
                # so each layer's vector tail hides under the other's matmuls.
                # x0h holds the current half; it is reloaded for half q+1 at
                # the end of the body, after its last reader (static SBUF AP,
                # dynamic DRAM offset only).
                NSH = NST // 2      # 32 steps per half
                NHALF = 2 * NCH
                x0h = x0pool.tile([P, MCH, HTJ], BF16)
                x1h = x0pool.tile([P, MCH, HTJ], BF16)

                def x1_from_h0(tag):
                    # x1 for the half just produced, straight from h0_loc
                    # (static APs; h1t round-trip not needed at all)
                    for m in range(MCH):
                        psx = psbig.tile([P, HTJ], F32, name=f"psx_{tag}{m}",
                                         tag="psx")
                        for k in range(KCH):
                            nc.tensor.matmul(
                                psx[:],
                                lhsT=wi1_sb[:, k, m * P : (m + 1) * P],
                                rhs=h0_loc[:, k, BC : (NSH + 1) * BC],
                                start=(k == 0),
                                stop=(k == KCH - 1),
                            )
                        if m % 2 == 0:
                            nc.scalar.activation(x1h[:, m, :], psx[:],
                                                 AF.Identity,
                                                 bias=b1_sb[:, m : m + 1],
                                                 scale=scale)
                        else:
                            nc.vector.tensor_scalar(
                                x1h[:, m, :], psx[:], scale,
                                b1_sb[:, m : m + 1], ALU.mult, ALU.add)

                def half_steps(run_l0, run_l1, tag):
                    for u in range(NSH):
                        if run_l0:
                            lstm_step(wh0_sb,
                                      x0h[:, :, u * BC : (u + 1) * BC],
                                      h0_loc, u,
                                      c0_ab[u % 2], c0_ab[(u + 1) % 2],
                                      f"{tag}0_{u}")
                        if run_l1:
                            lstm_step(wh1_sb,
                                      x1h[:, :, u * BC : (u + 1) * BC],
                                      h1_loc, u,
                                      c1_ab[u % 2], c1_ab[(u + 1) % 2],
                                      f"{tag}1_{u}", veng=nc.gpsimd)

                def l0_carry():
                    nc.vector.tensor_copy(
                        h0_loc[:, :, 0:BC],
                        h0_loc[:, :, NSH * BC : (NSH + 1) * BC])

                def l1_flush_h(hb):
                    nc.vector.tensor_copy(
                        h2t[:, :, ds(hb, HTJ)],
                        h1_loc[:, :, BC : (NSH + 1) * BC])
                    nc.vector.tensor_copy(
                        h1_loc[:, :, 0:BC],
                        h1_loc[:, :, NSH * BC : (NSH + 1) * BC])

                # prologue: half 0, L0 only; stage half 1; x1 for half 0
                for cpre in range(2):
                    nc.sync.dma_start(
                        x0h[:, :, cpre * TC : (cpre + 1) * TC],
                        x0t_dram[cpre : cpre + 1]
                        .rearrange("a p m t -> p m (a t)"))
                half_steps(True, False, "p")
                for cpre in range(2):
                    nc.sync.dma_start(
                        x0h[:, :, cpre * TC : (cpre + 1) * TC],
                        x0t_dram[2 + cpre : 3 + cpre]
                        .rearrange("a p m t -> p m (a t)"))
                x1_from_h0("p")
                l0_carry()

                with tc.For_i(1, NHALF, 1, staggered_reset=True) as q:
                    hb1 = nc.snap(q * HTJ - HTJ)          # L1 half base
                    half_steps(True, True, "s")
                    for cpre in range(2):
                        nc.sync.dma_start(
                            x0h[:, :, cpre * TC : (cpre + 1) * TC],
                            x0t_dram[ds(2 * q + 2 + cpre, 1)]
                            .rearrange("a p m t -> p m (a t)"))
                    x1_from_h0("s")
                    l0_carry()
                    l1_flush_h(hb1)

                # epilogue: L1 half NHALF-1 alone (x1h from last steady body)
                hbe = (NHALF - 1) * HTJ
                half_steps(False, True, "e")
                l1_flush_h(hbe)
            else:
                # ---- layer 0: 8 chunks of 64 steps ----
                with tc.For_i(0, NCH, 1, staggered_reset=True) as j:
                    jb = nc.snap(j * TJ)
                    nc.sync.dma_start(
                        x0t[1][:].rearrange("p m (a t) -> p m a t", t=TC),
                        x0t_dram[ds(GPC * j + 2, 2)]
                        .rearrange("a p m t -> p m a t"))
                    for u in range(NST // 2):
                        lstm_step(wh0_sb, x0_ap(u), h0_loc, u,
                                  c0_ab[u % 2], c0_ab[(u + 1) % 2], f"l0_{u}")
                    nc.sync.dma_start(
                        x0t[0][:].rearrange("p m (a t) -> p m a t", t=TC),
                        x0t_dram[ds(GPC * j + GPC, 2)]
                        .rearrange("a p m t -> p m a t"))
                    for u in range(NST // 2, NST):
                        lstm_step(wh0_sb, x0_ap(u), h0_loc, u,
                                  c0_ab[u % 2], c0_ab[(u + 1) % 2], f"l0_{u}")
                    l0_flush(jb)
                # ---- layer 1: 8 chunks, batched input matmul ----
                with tc.For_i(0, NCH, 1, staggered_reset=True) as j:
                    jb = nc.snap(j * TJ)
                    x1_batch(jb, "s")
                    for u in range(NST):
                        lstm_step(wh1_sb, x1t[:, :, u * BC : (u + 1) * BC],
                                  h1_loc, u,
                                  c1_ab[u % 2], c1_ab[(u + 1) % 2], f"l1_{u}")
                    l1_flush(jb)

            # ---- FC + sigmoid + store (static) ----
            for n in range(NT):
                obuf = work.tile([P, NM, 512], BF16, name=f"ob{n}", tag="ob",
                                 bufs=2)
                for m in range(NM):
                    psf = psbig.tile([P, 512], F32, name=f"psf{n}_{m}",
                                     tag="psf")
                    for k in range(KCH):
                        nc.tensor.matmul(
                            psf[:],
                            lhsT=fcw_sb[:, k, m * P : (m + 1) * P],
                            rhs=h2t[:, k, n * 512 : (n + 1) * 512],
                            start=(k == 0),
                            stop=(k == KCH - 1),
                        )
                    nc.scalar.activation(obuf[:, m, :], psf[:], AF.Sigmoid,
                                         bias=fcb_sb[:, m : m + 1])
                nc.sync.dma_start(out_d[:, n], obuf[:])

    if split:
        _split_waits(nc)
    return nc


_cache = {}


def _get_nc(use_fp8):
    ver = os.environ.get("DKT_V", "2")
    key = (ver, use_fp8, os.environ.get("DKT_IL", "1"))
    if key not in _cache:
        _cache[key] = (_build2 if ver == "2" else _build)(use_fp8)
    return _cache[key]


def kernel(skills, corrects, W_ih0, W_hh0, b_ih0, b_hh0,
           W_ih1, W_hh1, b_ih1, b_hh1, fc_W, fc_b):
    use_fp8 = os.environ.get("DKT_WDT", "bf16") == "fp8"
    scale = 64.0 if use_fp8 else 1.0
    np_wdt = np_fp8 if use_fp8 else np_bf16

    skills = np.asarray(skills, np.int32)
    corrects = np.asarray(corrects, np.int32)
    f32 = lambda x: np.asarray(x, np.float32).copy()
    W_ih0, W_hh0, W_ih1, W_hh1, fc_W = map(f32, (W_ih0, W_hh0, W_ih1, W_hh1, fc_W))
    b0 = f32(b_ih0) + f32(b_hh0)
    b1 = f32(b_ih1) + f32(b_hh1)
    fc_b = f32(fc_b)

    if os.environ.get("DKT_V", "2") == "2":
        # v2 computes every gate with one merged sigmoid; tanh(g) is
        # reconstructed as 2*sigmoid(2g)-1 with the 2x folded into the
        # g-gate rows here.
        gsl = slice(2 * H, 3 * H)
        W_ih0[gsl] *= 2
        W_hh0[gsl] *= 2
        W_ih1[gsl] *= 2
        W_hh1[gsl] *= 2
        b0[gsl] *= 2
        b1[gsl] *= 2

    table = np.ascontiguousarray(((W_ih0 + b0[:, None]).T * scale).astype(np_bf16))

    def kfmt(w, dt, sc=1.0):  # [G', 512] -> [128, 4, G'] lhsT chunks
        return np.ascontiguousarray(
            (w.T * sc).reshape(KCH, P, w.shape[0]).transpose(1, 0, 2).astype(dt))

    wh0 = kfmt(W_hh0, np_wdt, scale)
    wh1 = kfmt(W_hh1, np_wdt, scale)
    wi1 = kfmt(W_ih1, np_bf16)
    fcw = kfmt(fc_W, np_bf16)
    b1h = np.ascontiguousarray((b1 * scale).reshape(MCH, P).T.astype(np.float32))
    fcb = np.ascontiguousarray(fc_b.reshape(NSK // P, P).T.astype(np.float32))

    nc = _get_nc(use_fp8)

    in_maps = []
    for c in range(NCORES):
        sl = slice(c * BC, (c + 1) * BC)
        in_maps.append({
            "table": table, "wh0": wh0, "wh1": wh1, "wi1": wi1, "fcw": fcw,
            "b1": b1h, "fcb": fcb,
            "skills": np.ascontiguousarray(skills[sl]),
            "corrects": np.ascontiguousarray(corrects[sl]),
        })

    from concourse.bass_utils import run_bass_kernel_spmd
    trace = os.environ.get("DKT_TRACE", "0") == "1"
    if trace:
        import prof_shim
        prof_shim.install()
    fastprof = os.environ.get("DKT_FASTPROF", "0") == "1"
    if fastprof:
        import prof_shim
        with prof_shim.fast_profile() as fp:
            res = run_bass_kernel_spmd(nc, in_maps, core_ids=list(range(NCORES)),
                                       trace=False)
        fp.summarize()
    else:
        res = run_bass_kernel_spmd(nc, in_maps, core_ids=list(range(NCORES)),
                                   trace=trace)
    if trace:
        print(f"DKT exec_time_ns: {res.exec_time_ns}")
        kernel.last_result = res

    # out: [P, NT, NM, 512] per core, token = (s_in_chunk, b) -> [b, s, g]
    NT = S * BC // 512
    NM = NSK // P
    outs = []
    for r in res.results:
        a = np.asarray(r["out"], dtype=np.float32)
        a = (a.reshape(P, NT, NM, 64, BC)
             .transpose(4, 1, 3, 2, 0)
             .reshape(BC, S, NSK))
        outs.append(a)
    return np.concatenate(outs, axis=0)



# revision 16
# speedup vs baseline: 1.2797x; 1.2797x over previous
"""DKT model (2-layer LSTM + FC + sigmoid) as a Bass/Tile kernel for 8
Trainium2 NeuronCores, data-parallel over the batch dim (64 -> 8 per core).

Structure per core (everything "transposed": hidden/gate index on SBUF
partitions, batch on the free dim, so ACT/DVE use all 128 lanes):

  - One-hot @ W_ih0 is an embedding lookup: gather rows of
    (W_ih0 + b_ih0 + b_hh0)^T from a DRAM table via indirect DMA,
    DMA-transpose to gate-major layout, spill to DRAM, stream back
    during the recurrence.
  - LSTM recurrence (v2, default): gates accumulate in PSUM over 64
    [128x128]x[128x8] weight-stationary matmuls; x is added into PSUM in
    place (one DVE op); ALL FOUR gates go through a single merged ACT
    sigmoid straight out of PSUM (tanh(x) = 2*sigmoid(2x)-1, with the 2x
    folded into the g-gate rows of the weights host-side); the cell
    update is 6 small DVE ops; h lands in a local tile whose slots feed
    the next step's matmul rhs (static APs only on the PE).
  - The two layers run software-pipelined at half-chunk (256-token,
    32-step) granularity: iteration q interleaves L0 steps of half q
    with L1 steps of half q-1 per step, so each layer's serial
    ACT/DVE tail hides under the other layer's matmul stream.
  - Layer-1 input matmul X1 = h1 @ W_ih1^T + b1 batched per half chunk
    straight from the L0 local h tile at body end (N=256, static APs;
    no h1-history round trip); FC + sigmoid batched at the end, stored
    to DRAM in PE-native layout with large contiguous DMAs (the host
    untangles [g_lo, nchunk, m, tok] -> [b, s, g]; the naive transposed
    store degenerated to 4-byte DMA packets and serialized the kernel).
  Measured on HW: 4.43ms (from 54.9ms baseline). Known remaining gap:
  the Tile scheduler statically interleaves both layers' DVE chains, so
  the chain the next PE burst depends on finishes ~1us late per pair
  (~8.6us/pair vs ~5us ideal); forcing strict chain order with bufs=1
  scratch tiles made it WORSE (5.18ms) - each chain then waits out the
  other's full ACT round-trip latency.

Env knobs (defaults are the fast path): DKT_V=1 selects the legacy
build, DKT_IL=0 disables the layer interleave, DKT_WDT=fp8 switches the
recurrent weights to fp8 e4m3 with x64 scaling, DKT_S shortens the
sequence for dev testing, DKT_FASTPROF=1 captures an NTFF and prints
exec time via neuron-profile summary (dev only; needs prof_shim.py).
"""
import os
import numpy as np
import ml_dtypes

import concourse.bass as bass
import concourse.mybir as mybir
import concourse.tile as tile
import concourse.tile as tile_mod
from concourse.bass import ds
from concourse.tile_rust import add_dep_helper
from concourse.vector_clock import ScopedClock

BF16 = mybir.dt.bfloat16
FP8 = mybir.dt.float8e4
F32 = mybir.dt.float32
I32 = mybir.dt.int32
AF = mybir.ActivationFunctionType
ALU = mybir.AluOpType

np_bf16 = ml_dtypes.bfloat16
np_fp8 = ml_dtypes.float8_e4m3

P = 128
H = 512          # hidden
G = 2048         # 4*H gates
NSK = 1024       # n_skills
S = int(os.environ.get('DKT_S', '512'))  # seq len
B = 64           # full batch
NCORES = 8
BC = B // NCORES          # batch per core = 8
T = S * BC                # tokens per core = 4096
TC = P                    # tokens per gather chunk (= 16 steps)
NCHUNK = T // TC          # 32
KCH = 4                   # K chunks of 128 over hidden 512
MCH = 16                  # M chunks of 128 over gates 2048

# ---------------------------------------------------------------------------
# Toolchain workarounds (this walrus build rejects >1 semaphore wait per
# instruction, and Tile's exit drain / loop machinery emit several).

_MAX_WAITS = 1
_MAX_INC = 16
_wctr = [0]


def _split_waits(nc):
    for f in nc.m.functions:
        for bb in f.blocks:
            insts = bb.instructions
            i = 0
            while i < len(insts):
                inst = insts[i]
                si = inst.sync_info
                if si is not None and len(si.on_wait) > _MAX_WAITS:
                    waits = list(si.on_wait)
                    keep = waits[-_MAX_WAITS:]
                    extra = waits[:-_MAX_WAITS]
                    inst.sync_info = mybir.SyncInfo(
                        on_wait=keep, on_update=list(si.on_update)
                    )
                    si = inst.sync_info
                    pos = i
                    for j in range(0, len(extra), _MAX_WAITS):
                        _wctr[0] += 1
                        nop = mybir.InstNoOp(
                            name=f"wsplit_{_wctr[0]}", engine=inst.engine
                        )
                        nop.sync_info = mybir.SyncInfo(
                            on_wait=extra[j : j + _MAX_WAITS], on_update=[]
                        )
                        insts.insert(pos, nop)
                        pos += 1
                        i += 1
                # Non-NoOp instructions reject large sem-update immediates in
                # this walrus; move those updates intact onto an adjacent NoOp
                # (Tile's own stage-reset NoOps carry 1000+ values fine).
                # Never alter values: sem-wr-imm is an absolute write.
                # Same story for an update to a semaphore the instruction also
                # waits on (walrus 'no_semaphore_value_conflict').
                # DMA-ish instructions legitimately wait on and bump the same
                # FIFO semaphore (and need their completion update in place);
                # only compute ops trip the conflict check.
                is_compute = isinstance(
                    inst,
                    (
                        mybir.InstActivation,
                        mybir.InstTensorTensor,
                        mybir.InstTensorScalarPtr,
                        mybir.InstMatmult,
                        mybir.InstMemset,
                        mybir.InstTensorCopy,
                    ),
                )
                def _bad_update(u):
                    if (u.update_value or 0) > _MAX_INC:
                        return True
                    # compute ops only encode plain sem-inc; add-imm etc.
                    # must ride on a NoOp ('no_semaphore_value_conflict')
                    return is_compute and str(u.update_mode) != "sem-inc"

                if (
                    si is not None
                    and not isinstance(inst, mybir.InstNoOp)
                    and any(_bad_update(u) for u in si.on_update)
                ):
                    keep_ups, extras = [], []
                    for u in si.on_update:
                        if _bad_update(u):
                            extras.append(u)
                        else:
                            keep_ups.append(u)
                    inst.sync_info = mybir.SyncInfo(
                        on_wait=list(si.on_wait), on_update=keep_ups
                    )
                    is_branch = isinstance(
                        inst,
                        (
                            mybir.InstCompareAndBranch,
                            mybir.InstUnconditionalBranch,
                        ),
                    )
                    pos = i if is_branch else i + 1
                    for u in extras:
                        _wctr[0] += 1
                        nop = mybir.InstNoOp(
                            name=f"isplit_{_wctr[0]}", engine=inst.engine
                        )
                        nop.sync_info = mybir.SyncInfo(on_wait=[], on_update=[u])
                        insts.insert(pos, nop)
                        pos += 1
                        if is_branch:
                            i += 1
                i += 1


def _patched_drain_and_barrier(self, tick_clock, wait_clock):
    nc = self.nc
    drain_inst = nc.sync.drain()
    wait_clock.add_sem_waits(
        drain_inst.ins, ScopedClock({None: tick_clock.global_clock})
    )
    si = drain_inst.ins.sync_info
    if si is not None and len(si.on_wait) > 1:
        waits = list(si.on_wait)
        drain_inst.ins.sync_info = mybir.SyncInfo(
            on_wait=waits[:1], on_update=list(si.on_update)
        )
        for k in range(1, len(waits)):
            nop = nc.sync.nop()
            nop.ins.sync_info = mybir.SyncInfo(on_wait=[waits[k]], on_update=[])
    nc.all_engine_barrier()
    assert self.sems is not None
    popped = nc._tile_sem_poison_stack.pop()
    assert popped is self._sem_poison
    nc.clear_and_free_semaphores(list(self.sems.allocated().values()))
    nc.all_engine_barrier()


tile_mod.TileContext._drain_and_barrier = _patched_drain_and_barrier

# ---------------------------------------------------------------------------


def _build(use_fp8: bool, split: bool = True):
    wdt = FP8 if use_fp8 else BF16
    scale = 64.0 if use_fp8 else 1.0
    inv_scale = 1.0 / scale

    nc = bass.Bass()
    table_d = nc.dram_tensor("table", [G, G], BF16, kind="ExternalInput")
    wh0_d = nc.dram_tensor("wh0", [P, KCH, G], wdt, kind="ExternalInput")
    wh1_d = nc.dram_tensor("wh1", [P, KCH, G], wdt, kind="ExternalInput")
    wi1_d = nc.dram_tensor("wi1", [P, KCH, G], BF16, kind="ExternalInput")
    fcw_d = nc.dram_tensor("fcw", [P, KCH, NSK], BF16, kind="ExternalInput")
    b1_d = nc.dram_tensor("b1", [P, MCH], F32, kind="ExternalInput")
    fcb_d = nc.dram_tensor("fcb", [P, NSK // P], F32, kind="ExternalInput")
    sk_d = nc.dram_tensor("skills", [BC, S], I32, kind="ExternalInput")
    co_d = nc.dram_tensor("corrects", [BC, S], I32, kind="ExternalInput")
    # Output in PE-native layout [g_lo, nchunk, m, tok]; host untangles it.
    # (The old [b, s, g] transposed store degenerated to 4-byte DMA packets
    # and serialized the whole kernel on the sync DMA ring.)
    NT = T // 512
    NM = NSK // P
    out_d = nc.dram_tensor("out", [P, NT, NM, 512], BF16, kind="ExternalOutput")

    with tile.TileContext(nc) as tc:
        with tc.tile_pool(name="wpool", bufs=1) as wpool, \
             tc.tile_pool(name="hist", bufs=1) as hist, \
             tc.tile_pool(name="work", bufs=1) as work, \
             tc.tile_pool(name="dram", bufs=1, space="DRAM") as dpool, \
             tc.tile_pool(name="x0pool", bufs=1) as x0pool, \
             tc.tile_pool(name="psrec", bufs=2, space="PSUM") as psrec, \
             tc.tile_pool(name="psbig", bufs=2, space="PSUM") as psbig:

            # ---- resident weights ----
            wh0_sb = wpool.tile([P, KCH, G], wdt)
            nc.sync.dma_start(wh0_sb[:], wh0_d[:])
            wh1_sb = wpool.tile([P, KCH, G], wdt)
            nc.sync.dma_start(wh1_sb[:], wh1_d[:])
            wi1_sb = wpool.tile([P, KCH, G], BF16)
            nc.sync.dma_start(wi1_sb[:], wi1_d[:])
            fcw_sb = wpool.tile([P, KCH, NSK], BF16)
            nc.sync.dma_start(fcw_sb[:], fcw_d[:])
            b1_sb = wpool.tile([P, MCH], F32)
            nc.sync.dma_start(b1_sb[:], b1_d[:])
            fcb_sb = wpool.tile([P, NSK // P], F32)
            nc.sync.dma_start(fcb_sb[:], fcb_d[:])

            # ---- token-major idx = skills + 1024*(1 - corrects) ----
            sk_sb = work.tile([P, NCHUNK], I32)
            nc.sync.dma_start(
                sk_sb[:], sk_d[:].rearrange("b (c t) -> t b c", t=TC // BC))
            co_sb = work.tile([P, NCHUNK], I32)
            nc.sync.dma_start(
                co_sb[:], co_d[:].rearrange("b (c t) -> t b c", t=TC // BC))
            idx_sb = work.tile([P, NCHUNK], I32)
            nc.vector.tensor_scalar(
                idx_sb[:], co_sb[:], -1024, 1024, ALU.mult, ALU.add
            )
            nc.vector.tensor_tensor(
                out=idx_sb[:], in0=idx_sb[:], in1=sk_sb[:], op=ALU.add
            )

            # ---- gather + transpose + spill X0T to DRAM (static) ----
            x0t_dram = dpool.tile([NCHUNK + 2, P, MCH, TC], BF16)
            rows = [work.tile([P, G], BF16, name=f"rows{u}") for u in range(2)]
            x0st = [work.tile([P, MCH, TC], BF16, name=f"x0st{u}") for u in range(2)]
            for ch in range(NCHUNK):
                u = ch % 2
                nc.gpsimd.indirect_dma_start(
                    out=rows[u][:], out_offset=None, in_=table_d[:],
                    in_offset=bass.IndirectOffsetOnAxis(
                        ap=idx_sb[:, ch : ch + 1], axis=0
                    ),
                )
                for m in range(MCH):
                    nc.sync.dma_start_transpose(
                        x0st[u][:, m, :], rows[u][:, m * P : (m + 1) * P]
                    )
                nc.sync.dma_start(x0t_dram[ch], x0st[u][:])
            # pad chunks (read by the tail prefetch, never consumed)
            zpad = work.tile([P, MCH, TC], BF16)
            nc.vector.memset(zpad[:], 0.0)
            nc.sync.dma_start(x0t_dram[NCHUNK], zpad[:])
            nc.sync.dma_start(x0t_dram[NCHUNK + 1], zpad[:])

            # ---- histories (token-major, SBUF-resident) ----
            h1t = hist.tile([P, KCH, T], BF16)
            h2t = hist.tile([P, KCH, T], BF16)

            c_ab = [work.tile([P, KCH * BC], F32, name=f"c{u}") for u in range(2)]

            def lstm_step(w_sb, x_ap, h_loc, st, cprev, cnew, sname):
                """One step: gates = x + W_hh@h_prev; update c, h.
                x_ap: [P, MCH, BC] (pre-scaled by `scale`, bias folded);
                h_loc: local history [P, KCH, nsteps*BC + BC] with the
                previous body's last h in slots [0:BC]; step `st` reads
                slots [st*BC : st*BC+BC], writes the next BC."""
                psum = psrec.tile([P, MCH * BC], F32, name=f"ps_{sname}", tag="recps")
                for m in range(MCH):
                    for k in range(KCH):
                        nc.tensor.matmul(
                            psum[:, m * BC : (m + 1) * BC],
                            lhsT=w_sb[:, k, m * P : (m + 1) * P],
                            rhs=h_loc[:, k, st * BC : (st + 1) * BC],
                            start=(k == 0),
                            stop=(k == KCH - 1),
                        )
                gp = work.tile([P, MCH * BC], F32, name=f"gp_{sname}", tag="gp", bufs=2)
                nc.vector.tensor_tensor(
                    out=gp[:].rearrange("p (m b) -> p m b", b=BC),
                    in0=psum[:].rearrange("p (m b) -> p m b", b=BC),
                    in1=x_ap,
                    op=ALU.add,
                )
                ac = work.tile([P, MCH * BC], F32, name=f"ac_{sname}", tag="ac", bufs=2)
                nc.scalar.activation(ac[:, 0:64], gp[:, 0:64], AF.Sigmoid,
                                     scale=inv_scale)
                nc.scalar.activation(ac[:, 64:96], gp[:, 64:96], AF.Tanh,
                                     scale=inv_scale)
                nc.scalar.activation(ac[:, 96:128], gp[:, 96:128], AF.Sigmoid,
                                     scale=inv_scale)
                t1 = work.tile([P, KCH * BC], F32, name=f"t1_{sname}", tag="t1", bufs=2)
                nc.vector.tensor_tensor(out=t1[:], in0=ac[:, 32:64], in1=cprev[:],
                                        op=ALU.mult)
                t2 = work.tile([P, KCH * BC], F32, name=f"t2_{sname}", tag="t2", bufs=2)
                nc.vector.tensor_tensor(out=t2[:], in0=ac[:, 0:32], in1=ac[:, 64:96],
                                        op=ALU.mult)
                nc.vector.tensor_tensor(out=cnew[:], in0=t1[:], in1=t2[:], op=ALU.add)
                th = work.tile([P, KCH * BC], F32, name=f"th_{sname}", tag="th", bufs=2)
                nc.scalar.activation(th[:], cnew[:], AF.Tanh)
                nc.vector.tensor_tensor(
                    out=h_loc[:, :, (st + 1) * BC : (st + 2) * BC],
                    in0=ac[:, 96:128].rearrange("p (k b) -> p k b", b=BC),
                    in1=th[:].rearrange("p (k b) -> p k b", b=BC),
                    op=ALU.mult,
                )

            # ---- layer 0 recurrence: 16 iterations x 2 chunks x 16 steps ----
            nc.vector.memset(c_ab[0][:], 0.0)
            NST0 = 32  # steps per body
            h0_loc = work.tile([P, KCH, (NST0 + 1) * BC], BF16)
            nc.vector.memset(h0_loc[:, :, 0:BC], 0.0)
            x0t = [x0pool.tile([P, MCH, TC], BF16, name=f"x0t{u}") for u in range(2)]
            nc.sync.dma_start(x0t[0][:], x0t_dram[0])
            with tc.For_i(0, NCHUNK // 2, 1, staggered_reset=True) as i:
                nc.sync.dma_start(x0t[1][:], x0t_dram[ds(2 * i + 1, 1)]
                                  .rearrange("a p m t -> (a p) m t"))
                for half in range(2):
                    for u in range(16):
                        st = half * 16 + u  # step within the 2-chunk body
                        lstm_step(
                            wh0_sb,
                            x0t[half][:, :, u * BC : (u + 1) * BC],
                            h0_loc, st,
                            c_ab[st % 2], c_ab[(st + 1) % 2],
                            f"l0_{half}_{u}",
                        )
                nc.sync.dma_start(x0t[0][:], x0t_dram[ds(2 * i + 2, 1)]
                                  .rearrange("a p m t -> (a p) m t"))
                # flush this body's h into the history + carry last h to slot 0
                nc.vector.tensor_copy(
                    h1t[:, :, ds(i * (NST0 * BC), NST0 * BC)],
                    h0_loc[:, :, BC : (NST0 + 1) * BC],
                )
                nc.vector.tensor_copy(
                    h0_loc[:, :, 0:BC], h0_loc[:, :, NST0 * BC : (NST0 + 1) * BC]
                )

            # ---- layer 1: per 64-step chunk, batched X1 then recurrence ----
            nc.vector.memset(c_ab[0][:], 0.0)
            TJ = 512  # tokens per L1 chunk
            NST1 = TJ // BC  # 64 steps per body
            h1_loc = work.tile([P, KCH, (NST1 + 1) * BC], BF16)
            nc.vector.memset(h1_loc[:, :, 0:BC], 0.0)
            x1t = x0pool.tile([P, MCH, TJ], BF16)
            hx = work.tile([P, KCH, TJ], BF16)
            with tc.For_i(0, T // TJ, 1, staggered_reset=True) as j:
                jb = nc.snap(j * TJ)
                nc.vector.tensor_copy(hx[:], h1t[:, :, ds(jb, TJ)])
                for m in range(MCH):
                    psx = psbig.tile([P, TJ], F32, name=f"psx{m}", tag="psx")
                    for k in range(KCH):
                        nc.tensor.matmul(
                            psx[:],
                            lhsT=wi1_sb[:, k, m * P : (m + 1) * P],
                            rhs=hx[:, k, :],
                            start=(k == 0),
                            stop=(k == KCH - 1),
                        )
                    nc.scalar.activation(x1t[:, m, :], psx[:], AF.Identity,
                                         bias=b1_sb[:, m : m + 1], scale=scale)
                for u in range(NST1):
                    lstm_step(
                        wh1_sb,
                        x1t[:, :, u * BC : (u + 1) * BC],
                        h1_loc, u,
                        c_ab[u % 2], c_ab[(u + 1) % 2],
                        f"l1_{u}",
                    )
                nc.vector.tensor_copy(
                    h2t[:, :, ds(jb, TJ)],
                    h1_loc[:, :, BC : (NST1 + 1) * BC],
                )
                nc.vector.tensor_copy(
                    h1_loc[:, :, 0:BC], h1_loc[:, :, NST1 * BC : (NST1 + 1) * BC]
                )

            # ---- FC + sigmoid + store (static) ----
            for n in range(NT):
                obuf = work.tile([P, NM, 512], BF16, name=f"ob{n}", tag="ob",
                                 bufs=2)
                for m in range(NM):
                    psf = psbig.tile([P, 512], F32, name=f"psf{n}_{m}", tag="psf")
                    for k in range(KCH):
                        nc.tensor.matmul(
                            psf[:],
                            lhsT=fcw_sb[:, k, m * P : (m + 1) * P],
                            rhs=h2t[:, k, n * 512 : (n + 1) * 512],
                            start=(k == 0),
                            stop=(k == KCH - 1),
                        )
                    nc.scalar.activation(obuf[:, m, :], psf[:], AF.Sigmoid,
                                         bias=fcb_sb[:, m : m + 1])
                nc.sync.dma_start(out_d[:, n], obuf[:])

    if split:
        _split_waits(nc)
    return nc


def _build2(use_fp8: bool, split: bool = True):
    """v2: restructured recurrence.

    - all four gates through ONE merged sigmoid per step (tanh(x) =
      2*sigmoid(2x)-1; the 2x is folded into the g-gate rows of the
      weights/biases host-side), ACT reads gates straight from PSUM
    - x added into PSUM in place (single DVE op), no gp/ac/t1/t2/th tiles
    - h histories (h1t/h2t) double as the recurrence rhs via dynamic
      offsets: no h_loc, no history flush copies
    - gather scratch in a short-lived pool (SBUF reclaimed)
    - one For_i over 8 chunks of 64 steps per layer
    """
    wdt = FP8 if use_fp8 else BF16
    scale = 64.0 if use_fp8 else 1.0
    inv_scale = 1.0 / scale

    TJ = 512                  # tokens per chunk
    NST = TJ // BC            # 64 steps per chunk
    NCH = T // TJ             # 8 chunks
    GPC = TJ // TC            # 4 gather chunks per chunk

    nc = bass.Bass()
    table_d = nc.dram_tensor("table", [G, G], BF16, kind="ExternalInput")
    wh0_d = nc.dram_tensor("wh0", [P, KCH, G], wdt, kind="ExternalInput")
    wh1_d = nc.dram_tensor("wh1", [P, KCH, G], wdt, kind="ExternalInput")
    wi1_d = nc.dram_tensor("wi1", [P, KCH, G], BF16, kind="ExternalInput")
    fcw_d = nc.dram_tensor("fcw", [P, KCH, NSK], BF16, kind="ExternalInput")
    b1_d = nc.dram_tensor("b1", [P, MCH], F32, kind="ExternalInput")
    fcb_d = nc.dram_tensor("fcb", [P, NSK // P], F32, kind="ExternalInput")
    sk_d = nc.dram_tensor("skills", [BC, S], I32, kind="ExternalInput")
    co_d = nc.dram_tensor("corrects", [BC, S], I32, kind="ExternalInput")
    NT = T // 512
    NM = NSK // P
    out_d = nc.dram_tensor("out", [P, NT, NM, 512], BF16, kind="ExternalOutput")

    with tile.TileContext(nc) as tc:
        with tc.tile_pool(name="wpool", bufs=1) as wpool, \
             tc.tile_pool(name="hist", bufs=1) as hist, \
             tc.tile_pool(name="work", bufs=1) as work, \
             tc.tile_pool(name="dram", bufs=1, space="DRAM") as dpool, \
             tc.tile_pool(name="x0pool", bufs=1) as x0pool, \
             tc.tile_pool(name="psrec", bufs=2, space="PSUM") as psrec, \
             tc.tile_pool(name="psbig", bufs=2, space="PSUM") as psbig:

            # ---- resident weights ----
            wh0_sb = wpool.tile([P, KCH, G], wdt)
            nc.sync.dma_start(wh0_sb[:], wh0_d[:])
            wh1_sb = wpool.tile([P, KCH, G], wdt)
            nc.sync.dma_start(wh1_sb[:], wh1_d[:])
            wi1_sb = wpool.tile([P, KCH, G], BF16)
            nc.sync.dma_start(wi1_sb[:], wi1_d[:])
            fcw_sb = wpool.tile([P, KCH, NSK], BF16)
            nc.sync.dma_start(fcw_sb[:], fcw_d[:])
            b1_sb = wpool.tile([P, MCH], F32)
            nc.sync.dma_start(b1_sb[:], b1_d[:])
            fcb_sb = wpool.tile([P, NSK // P], F32)
            nc.sync.dma_start(fcb_sb[:], fcb_d[:])

            # ---- histories (filled chunk-wise from the local h tiles) ----
            h1t = hist.tile([P, KCH, T], BF16)
            h2t = hist.tile([P, KCH, T], BF16)

            # ---- gather + transpose + spill X0T to DRAM (static) ----
            x0t_dram = dpool.tile([NCHUNK + 2 * GPC, P, MCH, TC], BF16)
            with tc.tile_pool(name="gath", bufs=1) as gath:
                sk_sb = gath.tile([P, NCHUNK], I32)
                nc.sync.dma_start(
                    sk_sb[:], sk_d[:].rearrange("b (c t) -> t b c", t=TC // BC))
                co_sb = gath.tile([P, NCHUNK], I32)
                nc.sync.dma_start(
                    co_sb[:], co_d[:].rearrange("b (c t) -> t b c", t=TC // BC))
                idx_sb = gath.tile([P, NCHUNK], I32)
                nc.vector.tensor_scalar(
                    idx_sb[:], co_sb[:], -1024, 1024, ALU.mult, ALU.add
                )
                nc.vector.tensor_tensor(
                    out=idx_sb[:], in0=idx_sb[:], in1=sk_sb[:], op=ALU.add
                )
                rows = [gath.tile([P, G], BF16, name=f"rows{u}")
                        for u in range(2)]
                x0st = [gath.tile([P, MCH, TC], BF16, name=f"x0st{u}")
                        for u in range(2)]
                for ch in range(NCHUNK):
                    u = ch % 2
                    nc.gpsimd.indirect_dma_start(
                        out=rows[u][:], out_offset=None, in_=table_d[:],
                        in_offset=bass.IndirectOffsetOnAxis(
                            ap=idx_sb[:, ch : ch + 1], axis=0
                        ),
                    )
                    for m in range(MCH):
                        nc.sync.dma_start_transpose(
                            x0st[u][:, m, :], rows[u][:, m * P : (m + 1) * P]
                        )
                    nc.sync.dma_start(x0t_dram[ch], x0st[u][:])
                zpad = gath.tile([P, MCH, TC], BF16)
                nc.vector.memset(zpad[:], 0.0)
                for z in range(2 * GPC):
                    nc.sync.dma_start(x0t_dram[NCHUNK + z], zpad[:])

            # ---- per-layer state ----
            c0_ab = [work.tile([P, KCH * BC], F32, name=f"c0{u}")
                     for u in range(2)]
            c1_ab = [work.tile([P, KCH * BC], F32, name=f"c1{u}")
                     for u in range(2)]
            nc.vector.memset(c0_ab[0][:], 0.0)
            nc.vector.memset(c1_ab[0][:], 0.0)

            def lstm_step(w_sb, x_ap, h_loc, u, cprev, cnew, sname,
                          veng=None):
                """gates = W_hh @ h_prev (+x in psum); h -> h_loc slot u+1.

                veng picks the engine for the SBUF-only cell-update ops
                (default DVE). Running one layer's chain on GpSimd keeps
                the two interleaved layers' chains off each other's
                critical path (the PSUM add must stay on DVE: P2)."""
                if veng is None:
                    veng = nc.vector
                ps = psrec.tile([P, MCH * BC], F32, name=f"ps_{sname}",
                                tag="recps")
                for m in range(MCH):
                    for k in range(KCH):
                        nc.tensor.matmul(
                            ps[:, m * BC : (m + 1) * BC],
                            lhsT=w_sb[:, k, m * P : (m + 1) * P],
                            rhs=h_loc[:, k, u * BC : (u + 1) * BC],
                            start=(k == 0),
                            stop=(k == KCH - 1),
                        )
                nc.vector.tensor_tensor(
                    out=ps[:].rearrange("p (m b) -> p m b", b=BC),
                    in0=ps[:].rearrange("p (m b) -> p m b", b=BC),
                    in1=x_ap,
                    op=ALU.add,
                )
                ac = work.tile([P, MCH * BC], F32, name=f"ac_{sname}",
                               tag="ac", bufs=2)
                nc.scalar.activation(ac[:], ps[:], AF.Sigmoid, scale=inv_scale)
                # g' = tanh = 2*sig(2g)-1, in place
                veng.tensor_scalar(ac[:, 64:96], ac[:, 64:96], 2.0, -1.0,
                                   ALU.mult, ALU.add)
                tt = work.tile([P, 2 * KCH * BC], F32, name=f"tt_{sname}",
                               tag="tt", bufs=2)
                veng.tensor_tensor(out=tt[:, 0:32], in0=ac[:, 32:64],
                                   in1=cprev[:], op=ALU.mult)
                veng.tensor_tensor(out=tt[:, 32:64], in0=ac[:, 0:32],
                                   in1=ac[:, 64:96], op=ALU.mult)
                veng.tensor_tensor(out=cnew[:], in0=tt[:, 0:32],
                                   in1=tt[:, 32:64], op=ALU.add)
                sc = work.tile([P, KCH * BC], F32, name=f"sc_{sname}",
                               tag="sc", bufs=2)
                nc.scalar.activation(sc[:], cnew[:], AF.Sigmoid, scale=2.0)
                veng.tensor_scalar(sc[:], sc[:], 2.0, -1.0,
                                   ALU.mult, ALU.add)
                veng.tensor_tensor(
                    out=h_loc[:, :, (u + 1) * BC : (u + 2) * BC],
                    in0=ac[:, 96:128].rearrange("p (k b) -> p k b", b=BC),
                    in1=sc[:].rearrange("p (k b) -> p k b", b=BC),
                    op=ALU.mult,
                )

            # ---- local state + x buffers ----
            HTJ = TJ // 2             # tokens per x0 half-buffer
            h0_loc = work.tile([P, KCH, (NST + 1) * BC], BF16)
            nc.vector.memset(h0_loc[:, :, 0:BC], 0.0)
            h1_loc = work.tile([P, KCH, (NST + 1) * BC], BF16)
            nc.vector.memset(h1_loc[:, :, 0:BC], 0.0)
            interleave = os.environ.get("DKT_IL", "1") == "1"
            if not interleave:
                x0t = [x0pool.tile([P, MCH, HTJ], BF16, name=f"x0t{u}")
                       for u in range(2)]
                x1t = x0pool.tile([P, MCH, TJ], BF16)
                hx = x0pool.tile([P, KCH, TJ], BF16)
                nc.sync.dma_start(
                    x0t[0][:].rearrange("p m (a t) -> p m a t", t=TC),
                    x0t_dram[0:2].rearrange("a p m t -> p m a t"))

            def x0_ap(u):
                if u < NST // 2:
                    return x0t[0][:, :, u * BC : (u + 1) * BC]
                v = u - NST // 2
                return x0t[1][:, :, v * BC : (v + 1) * BC]

            def l0_flush(jb):
                nc.vector.tensor_copy(
                    h1t[:, :, ds(jb, TJ)], h0_loc[:, :, BC : (NST + 1) * BC])
                nc.vector.tensor_copy(
                    h0_loc[:, :, 0:BC],
                    h0_loc[:, :, NST * BC : (NST + 1) * BC])

            def l1_flush(jb):
                nc.vector.tensor_copy(
                    h2t[:, :, ds(jb, TJ)], h1_loc[:, :, BC : (NST + 1) * BC])
                nc.vector.tensor_copy(
                    h1_loc[:, :, 0:BC],
                    h1_loc[:, :, NST * BC : (NST + 1) * BC])

            def x1_batch(jb, tag):
                nc.vector.tensor_copy(hx[:], h1t[:, :, ds(jb, TJ)])
                for m in range(MCH):
                    psx = psbig.tile([P, TJ], F32, name=f"psx_{tag}{m}",
                                     tag="psx")
                    for k in range(KCH):
                        nc.tensor.matmul(
                            psx[:],
                            lhsT=wi1_sb[:, k, m * P : (m + 1) * P],
                            rhs=hx[:, k, :],
                            start=(k == 0),
                            stop=(k == KCH - 1),
                        )
                    if m % 2 == 0:
                        nc.scalar.activation(x1t[:, m, :], psx[:], AF.Identity,
                                             bias=b1_sb[:, m : m + 1],
                                             scale=scale)
                    else:
                        nc.vector.tensor_scalar(
                            x1t[:, m, :], psx[:], scale,
                            b1_sb[:, m : m + 1], ALU.mult, ALU.add)

            if interleave:
                # Half-chunk (HTJ=256 tok, 32 steps) software pipeline:
                # iteration q runs L0 half q and L1 half q-1 step-interleaved
                # so each layer's vector tail hides under the other's matmuls.
                # x0h holds the current half; it is reloaded for half q+1 at
                # the end of the body, after its last reader (static SBUF AP,
                # dynamic DRAM offset only).
                NSH = NST // 2      # 32 steps per half
                NHALF = 2 * NCH
                x0h = x0pool.tile([P, MCH, HTJ], BF16)
                x1h = x0pool.tile([P, MCH, HTJ], BF16)

                def x1_from_h0(tag):
                    # x1 for the half just produced, straight from h0_loc
                    # (static APs; h1t round-trip not needed at all)
                    for m in range(MCH):
                        psx = psbig.tile([P, HTJ], F32, name=f"psx_{tag}{m}",
                                         tag="psx")
                        for k in range(KCH):
                            nc.tensor.matmul(
                                psx[:],
                                lhsT=wi1_sb[:, k, m * P : (m + 1) * P],
                                rhs=h0_loc[:, k, BC : (NSH + 1) * BC],
                                start=(k == 0),
                                stop=(k == KCH - 1),
                            )
                        if m % 2 == 0:
                            nc.scalar.activation(x1h[:, m, :], psx[:],
                                                 AF.Identity,
                                                 bias=b1_sb[:, m : m + 1],
                                                 scale=scale)
                        else:
                            nc.vector.tensor_scalar(
                                x1h[:, m, :], psx[:], scale,
                                b1_sb[:, m : m + 1], ALU.mult, ALU.add)

                def half_steps(run_l0, run_l1, tag):
                    for u in range(NSH):
                        if run_l0:
                            lstm_step(wh0_sb,
                                      x0h[:, :, u * BC : (u + 1) * BC],
                                      h0_loc, u,
                                      c0_ab[u % 2], c0_ab[(u + 1) % 2],
                                      f"{tag}0_{u}")
                        if run_l1:
                            lstm_step(wh1_sb,
                                      x1h[:, :, u * BC : (u + 1) * BC],
                                      h1_loc, u,
                                      c1_ab[u % 2], c1_ab[(u + 1) % 2],
                                      f"{tag}1_{u}", veng=nc.gpsimd)

                def l0_carry():
                    nc.vector.tensor_copy(
                        h0_loc[:, :, 0:BC],
                        h0_loc[:, :, NSH * BC : (NSH + 1) * BC])

                def l1_flush_h(hb):
                    nc.vector.tensor_copy(
                        h2t[:, :, ds(hb, HTJ)],
                        h1_loc[:, :, BC : (NSH + 1) * BC])
                    nc.vector.tensor_copy(
                        h1_loc[:, :, 0:BC],
                        h1_loc[:, :, NSH * BC : (NSH + 1) * BC])

                # prologue: half 0, L0 only; stage half 1; x1 for half 0
                for cpre in range(2):
                    nc.sync.dma_start(
                        x0h[:, :, cpre * TC : (cpre + 1) * TC],
                        x0t_dram[cpre : cpre + 1]
                        .rearrange("a p m t -> p m (a t)"))
                half_steps(True, False, "p")
                for cpre in range(2):
                    nc.sync.dma_start(
                        x0h[:, :, cpre * TC : (cpre + 1) * TC],
                        x0t_dram[2 + cpre : 3 + cpre]
                        .rearrange("a p m t -> p m (a t)"))
                x1_from_h0("p")
                l0_carry()

                with tc.For_i(1, NHALF, 1, staggered_reset=True) as q:
                    hb1 = nc.snap(q * HTJ - HTJ)          # L1 half base
                    half_steps(True, True, "s")
                    for cpre in range(2):
                        nc.sync.dma_start(
                            x0h[:, :, cpre * TC : (cpre + 1) * TC],
                            x0t_dram[ds(2 * q + 2 + cpre, 1)]
                            .rearrange("a p m t -> p m (a t)"))
                    x1_from_h0("s")
                    l0_carry()
                    l1_flush_h(hb1)

                # epilogue: L1 half NHALF-1 alone (x1h from last steady body)
                hbe = (NHALF - 1) * HTJ
                half_steps(False, True, "e")
                l1_flush_h(hbe)
            else:
                # ---- layer 0: 8 chunks of 64 steps ----
                with tc.For_i(0, NCH, 1, staggered_reset=True) as j:
                    jb = nc.snap(j * TJ)
                    nc.sync.dma_start(
                        x0t[1][:].rearrange("p m (a t) -> p m a t", t=TC),
                        x0t_dram[ds(GPC * j + 2, 2)]
                        .rearrange("a p m t -> p m a t"))
                    for u in range(NST // 2):
                        lstm_step(wh0_sb, x0_ap(u), h0_loc, u,
                                  c0_ab[u % 2], c0_ab[(u + 1) % 2], f"l0_{u}")
                    nc.sync.dma_start(
                        x0t[0][:].rearrange("p m (a t) -> p m a t", t=TC),
                        x0t_dram[ds(GPC * j + GPC, 2)]
                        .rearrange("a p m t -> p m a t"))
                    for u in range(NST // 2, NST):
                        lstm_step(wh0_sb, x0_ap(u), h0_loc, u,
                                  c0_ab[u % 2], c0_ab[(u + 1) % 2], f"l0_{u}")
                    l0_flush(jb)
                # ---- layer 1: 8 chunks, batched input matmul ----
                with tc.For_i(0, NCH, 1, staggered_reset=True) as j:
                    jb = nc.snap(j * TJ)
                    x1_batch(jb, "s")
                    for u in range(NST):
                        lstm_step(wh1_sb, x1t[:, :, u * BC : (u + 1) * BC],
                                  h1_loc, u,
                                  c1_ab[u % 2], c1_ab[(u + 1) % 2], f"l1_{u}")
                    l1_flush(jb)

            # ---- FC + sigmoid + store (static) ----
            for n in range(NT):
                obuf = work.tile([P, NM, 512], BF16, name=f"ob{n}", tag="ob",
                                 bufs=2)
                for m in range(NM):
                    psf = psbig.tile([P, 512], F32, name=f"psf{n}_{m}",
                                     tag="psf")
                    for k in range(KCH):
                        nc.tensor.matmul(
                            psf[:],
                            lhsT=fcw_sb[:, k, m * P : (m + 1) * P],
                            rhs=h2t[:, k, n * 512 : (n + 1) * 512],
                            start=(k == 0),
                            stop=(k == KCH - 1),
                        )
                    nc.scalar.activation(obuf[:, m, :], psf[:], AF.Sigmoid,
                                         bias=fcb_sb[:, m : m + 1])
                nc.sync.dma_start(out_d[:, n], obuf[:])

    if split:
        _split_waits(nc)
    return nc


def _build3(split: bool = True):
    """v3: chain-latency-focused restructure of the v2 interleave.

    Per the profile, the recurrence matmul bursts are PE-issue-bound
    (~27ns per LDW+MM pair, clock/dtype independent) and the per-step
    LSTM cell-update chain (~3us of mostly-exposed ACT/DVE latency) is
    the critical path. Changes vs v2:

    - x is accumulated into the gates PSUM by the PE itself: one extra
      matmul per step with lhsT = I_128 and rhs = the step's x slice
      (start=False accumulate). Kills the per-step DVE PSUM-add, which
      both sat on the chain AND head-of-line-blocked the other layer's
      chain ops on the vector queue.
    - tanh(c) via AF.Tanh directly (one ACT op) instead of
      sigmoid(2c) + DVE 2x-1 fixup (two ops + a hop).
    - Each layer's 5 cell-update DVE ops run on ONE engine (L0 vector,
      L1 gpsimd); the only cross-engine hops left are PE->ACT->veng
      ->ACT->veng.
    - The gather pipeline is restructured to overlap the body: the 32
      indirect gathers bounce through SBUF into a DRAM row cache in the
      prologue (scalar DMA ring, drains fast); the gate-major transpose
      happens in-loop via dma_start_transpose straight from DRAM into
      two SBUF chunk buffers (even/odd), staged half a chunk ahead.
      The old dedicated gather+transpose+spill phase (~0.44ms) is gone.
    - FC runs per half-chunk inside the loop on h1_loc directly (h1t /
      h2t SBUF histories and the 0.2ms FC tail are gone); output in
      PE-native layout [g_lo, half, m, tok].
    - No fp8 (measured: zero effect - the bursts are issue-bound).
    """
    NSH = 32                  # steps per half
    HTJ = NSH * BC            # 256 tokens per half
    NHALF = T // HTJ          # 16
    NM = NSK // P             # 8

    nc = bass.Bass()
    table_d = nc.dram_tensor("table", [G, G], BF16, kind="ExternalInput")
    wh0_d = nc.dram_tensor("wh0", [P, KCH, G], BF16, kind="ExternalInput")
    wh1_d = nc.dram_tensor("wh1", [P, KCH, G], BF16, kind="ExternalInput")
    wi1_d = nc.dram_tensor("wi1", [P, KCH, G], BF16, kind="ExternalInput")
    fcw_d = nc.dram_tensor("fcw", [P, KCH, NSK], BF16, kind="ExternalInput")
    b1_d = nc.dram_tensor("b1", [P, MCH], F32, kind="ExternalInput")
    fcb_d = nc.dram_tensor("fcb", [P, NM], F32, kind="ExternalInput")
    ident_d = nc.dram_tensor("ident", [P, P], BF16, kind="ExternalInput")
    sk_d = nc.dram_tensor("skills", [BC, S], I32, kind="ExternalInput")
    co_d = nc.dram_tensor("corrects", [BC, S], I32, kind="ExternalInput")
    out_d = nc.dram_tensor("out", [P, NHALF, NM, HTJ], BF16,
                           kind="ExternalOutput")

    with tile.TileContext(nc) as tc:
        with tc.tile_pool(name="wpool", bufs=1) as wpool, \
             tc.tile_pool(name="work", bufs=1) as work, \
             tc.tile_pool(name="dram", bufs=1, space="DRAM") as dpool, \
             tc.tile_pool(name="x0pool", bufs=1) as x0pool, \
             tc.tile_pool(name="psrec", bufs=2, space="PSUM") as psrec, \
             tc.tile_pool(name="psbig", bufs=2, space="PSUM") as psbig:

            # ---- resident weights ----
            wh0_sb = wpool.tile([P, KCH, G], BF16)
            nc.sync.dma_start(wh0_sb[:], wh0_d[:])
            wh1_sb = wpool.tile([P, KCH, G], BF16)
            nc.sync.dma_start(wh1_sb[:], wh1_d[:])
            wi1_sb = wpool.tile([P, KCH, G], BF16)
            nc.sync.dma_start(wi1_sb[:], wi1_d[:])
            fcw_sb = wpool.tile([P, KCH, NSK], BF16)
            nc.sync.dma_start(fcw_sb[:], fcw_d[:])
            b1_sb = wpool.tile([P, MCH], F32)
            nc.sync.dma_start(b1_sb[:], b1_d[:])
            fcb_sb = wpool.tile([P, NM], F32)
            nc.sync.dma_start(fcb_sb[:], fcb_d[:])
            ident_sb = wpool.tile([P, P], BF16)
            nc.sync.dma_start(ident_sb[:], ident_d[:])

            # ---- token-major idx = skills + 1024*(1 - corrects) ----
            sk_sb = work.tile([P, NCHUNK], I32)
            nc.sync.dma_start(
                sk_sb[:], sk_d[:].rearrange("b (c t) -> t b c", t=TC // BC))
            co_sb = work.tile([P, NCHUNK], I32)
            nc.sync.dma_start(
                co_sb[:], co_d[:].rearrange("b (c t) -> t b c", t=TC // BC))
            idx_sb = work.tile([P, NCHUNK], I32)
            nc.vector.tensor_scalar(
                idx_sb[:], co_sb[:], -1024, 1024, ALU.mult, ALU.add)
            nc.vector.tensor_tensor(
                out=idx_sb[:], in0=idx_sb[:], in1=sk_sb[:], op=ALU.add)

            # ---- gather all chunks to a DRAM row cache (token-major) ----
            # indirect gathers must land in SBUF; bounce via 2 row bufs,
            # spills on the scalar DMA ring so the in-loop sync-ring
            # traffic (transposes, stores) never queues behind them.
            # +2 junk chunks: the loop prefetch at q=NHALF-1 reads them.
            rows_dram = dpool.tile([NCHUNK + 2, P, G], BF16)
            rows = [work.tile([P, G], BF16, name=f"rows{u}") for u in range(2)]
            zrow = work.tile([P, G], BF16)
            nc.vector.memset(zrow[:], 0.0)
            nc.scalar.dma_start(rows_dram[NCHUNK], zrow[:])
            nc.scalar.dma_start(rows_dram[NCHUNK + 1], zrow[:])
            for ch in range(NCHUNK):
                u = ch % 2
                nc.gpsimd.indirect_dma_start(
                    out=rows[u][:], out_offset=None, in_=table_d[:],
                    in_offset=bass.IndirectOffsetOnAxis(
                        ap=idx_sb[:, ch : ch + 1], axis=0),
                )
                nc.scalar.dma_start(rows_dram[ch], rows[u][:])

            # ---- x0 chunk buffers (gate-major), staged from rows_dram ----
            # buf 0 holds even chunks, buf 1 odd chunks: static APs, each
            # restaged every half right after its last reader. The DMA
            # transpose can't take a register-offset DRAM source, so the
            # dynamic row-cache read bounces through an SBUF row buffer
            # (separate from the prologue gather bounce bufs - sharing
            # those would chain a WAR wait on the whole prologue drain).
            x0b = [x0pool.tile([P, MCH, TC], BF16, name=f"x0b{u}")
                   for u in range(2)]
            xrow = [x0pool.tile([P, G], BF16, name=f"xrow{u}")
                    for u in range(2)]

            def stage_chunk(ch, buf):
                # ch: python int or For_i expression (dynamic DRAM offset)
                src = rows_dram[ch] if isinstance(ch, int) else \
                    rows_dram[ds(ch, 1)].rearrange("a p g -> (a p) g")
                nc.sync.dma_start(xrow[buf][:], src)
                for m in range(MCH):
                    nc.sync.dma_start_transpose(
                        x0b[buf][:, m, :], xrow[buf][:, m * P : (m + 1) * P])

            # ---- per-layer state ----
            c0_ab = [work.tile([P, KCH * BC], F32, name=f"c0{u}")
                     for u in range(2)]
            c1_ab = [work.tile([P, KCH * BC], F32, name=f"c1{u}")
                     for u in range(2)]
            nc.vector.memset(c0_ab[0][:], 0.0)
            nc.vector.memset(c1_ab[0][:], 0.0)

            # local h: slot s in [0,NSH] = h after step s-1; slot NSH wraps
            # to feed the next half's step 0 (no carry copy needed).
            h0_loc = work.tile([P, KCH, (NSH + 1) * BC], BF16)
            nc.vector.memset(h0_loc[:, :, 0:BC], 0.0)
            h1_loc = work.tile([P, KCH, (NSH + 1) * BC], BF16)
            # L1's first step (iteration q=1, u=0) reads slot NSH
            nc.vector.memset(h1_loc[:, :, NSH * BC : (NSH + 1) * BC], 0.0)
            x1h = x0pool.tile([P, MCH, HTJ], BF16)

            # ACT-stream order pin: the sim-model-driven scheduler orders
            # each engine's instruction stream by its own timing estimates;
            # on HW that put L1's sigmoid ahead of L0's tanh(c), head-of-
            # line-blocking the L0 chain ~1.3us/pair. Chain every ACT op
            # behind its intended predecessor with a nosync edge (stream
            # order only, no runtime semaphore cost).
            act_chain = [None]

            def act_ordered(inst):
                if act_chain[0] is not None:
                    add_dep_helper(inst.ins, act_chain[0].ins, sync=False,
                                   reason="act stream order")
                act_chain[0] = inst

            def lstm_step(w_sb, x_ap, h_loc, rslot, wslot, cprev, cnew,
                          sname, veng):
                """gates = W_hh @ h[rslot] + x (x via PE identity-matmul);
                cell update entirely on `veng` + 2 ACT ops."""
                ps = psrec.tile([P, MCH * BC], F32, name=f"ps_{sname}",
                                tag="recps")
                for m in range(MCH):
                    for k in range(KCH):
                        nc.tensor.matmul(
                            ps[:, m * BC : (m + 1) * BC],
                            lhsT=w_sb[:, k, m * P : (m + 1) * P],
                            rhs=h_loc[:, k, rslot * BC : (rslot + 1) * BC],
                            start=(k == 0),
                            stop=False,
                            skip_group_check=True,
                        )
                nc.tensor.matmul(
                    ps[:],
                    lhsT=ident_sb[:],
                    rhs=x_ap,
                    start=False,
                    stop=True,
                    skip_group_check=True,
                )
                ac = work.tile([P, MCH * BC], F32, name=f"ac_{sname}",
                               tag="ac", bufs=2)
                act_ordered(nc.scalar.activation(ac[:], ps[:], AF.Sigmoid))
                # g' = tanh(g) = 2*sig(2g)-1 (2x folded into weights)
                veng.tensor_scalar(ac[:, 64:96], ac[:, 64:96], 2.0, -1.0,
                                   ALU.mult, ALU.add)
                tt = work.tile([P, 2 * KCH * BC], F32, name=f"tt_{sname}",
                               tag="tt", bufs=2)
                veng.tensor_tensor(out=tt[:, 0:32], in0=ac[:, 32:64],
                                   in1=cprev[:], op=ALU.mult)
                veng.tensor_tensor(out=tt[:, 32:64], in0=ac[:, 0:32],
                                   in1=ac[:, 64:96], op=ALU.mult)
                veng.tensor_tensor(out=cnew[:], in0=tt[:, 0:32],
                                   in1=tt[:, 32:64], op=ALU.add)
                th = work.tile([P, KCH * BC], F32, name=f"th_{sname}",
                               tag="th", bufs=2)
                act_ordered(nc.scalar.activation(th[:], cnew[:], AF.Tanh))
                veng.tensor_tensor(
                    out=h_loc[:, :, wslot * BC : (wslot + 1) * BC],
                    in0=ac[:, 96:128].rearrange("p (k b) -> p k b", b=BC),
                    in1=th[:].rearrange("p (k b) -> p k b", b=BC),
                    op=ALU.mult,
                )

            def l0_step(u, first_half=False):
                buf = u // 16
                t0 = u % 16
                rslot = u if (u > 0 or first_half) else NSH
                lstm_step(
                    wh0_sb,
                    x0b[buf][:, :, t0 * BC : (t0 + 1) * BC],
                    h0_loc, rslot, u + 1,
                    c0_ab[u % 2], c0_ab[(u + 1) % 2],
                    f"l0_{u}", nc.vector,
                )

            def l1_step(u):
                rslot = u if u > 0 else NSH
                lstm_step(
                    wh1_sb,
                    x1h[:, :, u * BC : (u + 1) * BC],
                    h1_loc, rslot, u + 1,
                    c1_ab[u % 2], c1_ab[(u + 1) % 2],
                    f"l1_{u}", nc.gpsimd,
                )

            def x1_lump(tag):
                # x1 = W_ih1 @ h0(this half) + b1, batched over 256 tokens
                for m in range(MCH):
                    psx = psbig.tile([P, HTJ], F32, name=f"psx_{tag}{m}",
                                     tag="psx")
                    for k in range(KCH):
                        nc.tensor.matmul(
                            psx[:],
                            lhsT=wi1_sb[:, k, m * P : (m + 1) * P],
                            rhs=h0_loc[:, k, BC : (NSH + 1) * BC],
                            start=(k == 0),
                            stop=(k == KCH - 1),
                        )
                    if m % 2 == 0:
                        act_ordered(nc.scalar.activation(
                            x1h[:, m, :], psx[:], AF.Identity,
                            bias=b1_sb[:, m : m + 1]))
                    else:
                        nc.vector.tensor_scalar(
                            x1h[:, m, :], psx[:], 1.0,
                            b1_sb[:, m : m + 1], ALU.mult, ALU.add)

            def fc_lump(tag, out_ap):
                # FC + sigmoid on h1(this half), store PE-native
                obuf = work.tile([P, NM, HTJ], BF16, name=f"ob_{tag}",
                                 tag="ob", bufs=2)
                for m in range(NM):
                    psf = psbig.tile([P, HTJ], F32, name=f"psf_{tag}{m}",
                                     tag="psf")
                    for k in range(KCH):
                        nc.tensor.matmul(
                            psf[:],
                            lhsT=fcw_sb[:, k, m * P : (m + 1) * P],
                            rhs=h1_loc[:, k, BC : (NSH + 1) * BC],
                            start=(k == 0),
                            stop=(k == KCH - 1),
                        )
                    act_ordered(nc.scalar.activation(
                        obuf[:, m, :], psf[:], AF.Sigmoid,
                        bias=fcb_sb[:, m : m + 1]))
                nc.sync.dma_start(out_ap, obuf[:])

            # ---- prologue: half 0, L0 only ----
            # buf1's chunk for each loop iteration is staged at the TOP of
            # that iteration (consumed by its steps 16-31, ~80us later):
            # staging it at the previous iteration's END put 16 serialized
            # ~1.24us DMA transposes after the last compute, and the loop
            # barrier exposed the whole ~15us tail on every iteration.
            stage_chunk(0, 0)
            stage_chunk(1, 1)
            for u in range(NSH):
                l0_step(u, first_half=True)
                if u == 15:
                    stage_chunk(2, 0)
            x1_lump("p")

            # ---- steady loop: L0 half q + L1 half q-1 ----
            with tc.For_i(1, NHALF, 1, staggered_reset=True) as q:
                act_chain[0] = None  # don't chain across the block boundary
                stage_chunk(2 * q + 1, 1)
                for u in range(NSH):
                    l0_step(u)
                    l1_step(u)
                    if u == 15:
                        stage_chunk(2 * q + 2, 0)
                x1_lump("s")
                fc_lump("s",
                        out_d[:, ds(q - 1, 1)]
                        .rearrange("p a m t -> p (a m) t"))

            # ---- epilogue: L1 half NHALF-1 + its FC ----
            act_chain[0] = None
            for u in range(NSH):
                l1_step(u)
            fc_lump("e", out_d[:, NHALF - 1])

    if split:
        _split_waits(nc)
    return nc


_cache = {}


def _get_nc(use_fp8):
    ver = os.environ.get("DKT_V", "3")
    key = (ver, use_fp8, os.environ.get("DKT_IL", "1"))
    if key not in _cache:
        if ver == "3":
            _cache[key] = _build3()
        else:
            _cache[key] = (_build2 if ver == "2" else _build)(use_fp8)
    return _cache[key]


def kernel(skills, corrects, W_ih0, W_hh0, b_ih0, b_hh0,
           W_ih1, W_hh1, b_ih1, b_hh1, fc_W, fc_b):
    ver = os.environ.get("DKT_V", "3")
    use_fp8 = ver != "3" and os.environ.get("DKT_WDT", "bf16") == "fp8"
    scale = 64.0 if use_fp8 else 1.0
    np_wdt = np_fp8 if use_fp8 else np_bf16

    skills = np.asarray(skills, np.int32)
    corrects = np.asarray(corrects, np.int32)
    f32 = lambda x: np.asarray(x, np.float32).copy()
    W_ih0, W_hh0, W_ih1, W_hh1, fc_W = map(f32, (W_ih0, W_hh0, W_ih1, W_hh1, fc_W))
    b0 = f32(b_ih0) + f32(b_hh0)
    b1 = f32(b_ih1) + f32(b_hh1)
    fc_b = f32(fc_b)

    if ver in ("2", "3"):
        # v2 computes every gate with one merged sigmoid; tanh(g) is
        # reconstructed as 2*sigmoid(2g)-1 with the 2x folded into the
        # g-gate rows here.
        gsl = slice(2 * H, 3 * H)
        W_ih0[gsl] *= 2
        W_hh0[gsl] *= 2
        W_ih1[gsl] *= 2
        W_hh1[gsl] *= 2
        b0[gsl] *= 2
        b1[gsl] *= 2

    table = np.ascontiguousarray(((W_ih0 + b0[:, None]).T * scale).astype(np_bf16))

    def kfmt(w, dt, sc=1.0):  # [G', 512] -> [128, 4, G'] lhsT chunks
        return np.ascontiguousarray(
            (w.T * sc).reshape(KCH, P, w.shape[0]).transpose(1, 0, 2).astype(dt))

    wh0 = kfmt(W_hh0, np_wdt, scale)
    wh1 = kfmt(W_hh1, np_wdt, scale)
    wi1 = kfmt(W_ih1, np_bf16)
    fcw = kfmt(fc_W, np_bf16)
    b1h = np.ascontiguousarray((b1 * scale).reshape(MCH, P).T.astype(np.float32))
    fcb = np.ascontiguousarray(fc_b.reshape(NSK // P, P).T.astype(np.float32))

    nc = _get_nc(use_fp8)

    in_maps = []
    for c in range(NCORES):
        sl = slice(c * BC, (c + 1) * BC)
        im = {
            "table": table, "wh0": wh0, "wh1": wh1, "wi1": wi1, "fcw": fcw,
            "b1": b1h, "fcb": fcb,
            "skills": np.ascontiguousarray(skills[sl]),
            "corrects": np.ascontiguousarray(corrects[sl]),
        }
        if ver == "3":
            im["ident"] = np.eye(P, dtype=np_bf16)
        in_maps.append(im)

    from concourse.bass_utils import run_bass_kernel_spmd
    trace = os.environ.get("DKT_TRACE", "0") == "1"
    if trace:
        import prof_shim
        prof_shim.install()
    fastprof = os.environ.get("DKT_FASTPROF", "0") == "1"
    if fastprof:
        import prof_shim
        with prof_shim.fast_profile() as fp:
            res = run_bass_kernel_spmd(nc, in_maps, core_ids=list(range(NCORES)),
                                       trace=False)
        fp.summarize()
    else:
        res = run_bass_kernel_spmd(nc, in_maps, core_ids=list(range(NCORES)),
                                   trace=trace)
    if trace:
        print(f"DKT exec_time_ns: {res.exec_time_ns}")
        kernel.last_result = res

    NM = NSK // P
    outs = []
    if ver == "3":
        # out: [P, NHALF=16, NM, 256] per core; token = (s_in_half, b)
        for r in res.results:
            a = np.asarray(r["out"], dtype=np.float32)
            a = (a.reshape(P, 16, NM, 32, BC)
                 .transpose(4, 1, 3, 2, 0)
                 .reshape(BC, S, NSK))
            outs.append(a)
    else:
        # out: [P, NT, NM, 512] per core, token = (s_in_chunk, b)
        NT = S * BC // 512
        for r in res.results:
            a = np.asarray(r["out"], dtype=np.float32)
            a = (a.reshape(P, NT, NM, 64, BC)
                 .transpose(4, 1, 3, 2, 0)
                 .reshape(BC, S, NSK))
            outs.append(a)
    return np.concatenate(outs, axis=0)

